# revision 73
# baseline (speedup 1.0000x reference)
"""Trainium2 Bass kernel for nn_Block_87428354277599 (sinkhorn-attention transformer block).

Self-contained: hardcodes shapes/sharding. kernel(**inputs) -> (2, 2048, 384) f32.

Sharding (8 cores, SPMD):
- 12 (batch, head) units padded to 16 slots: every core runs 2 attention slots
  (cores 4-7's slot 1 gets zero weights; its junk output is never consumed).
- LN1/LN2 are folded into the QKV / MLP matmuls via host-precomputed weight folds
  plus rank-1 corrections (mu and t-column terms) accumulated on the PE.
- Sinkhorn on the row-softmaxed causal attention == multiplicative matrix scaling
  of S = exp(P). S-1 is lower-triangular, so only the lower triangle (S' = S-1)
  is stored SBUF-resident in both layouts (S' f32, S'^T bf16); the all-ones part
  of S becomes global-sum corrections (kept f32). All matvecs run on the PE.
- y^T slices are exchanged with one AllToAll (each sender duplicates its slices
  into both batch shard groups; receivers mask the wrong batch via zeroed halves
  of the duplicated proj weights). proj+LN2+MLP run row-sharded (512 rows/core).
"""

import numpy as np
import ml_dtypes

BF16NP = ml_dtypes.bfloat16

import concourse.bacc as bacc
import concourse.mybir as mybir
from concourse.tile import TileContext
from concourse.bass_utils import run_bass_kernel_spmd

F32 = mybir.dt.float32
BF16 = mybir.dt.bfloat16
F32R = mybir.dt.float32r
AF = mybir.ActivationFunctionType
ALU = mybir.AluOpType
AXX = mybir.AxisListType.X

B, T, C, H, HD = 2, 2048, 384, 6, 64
CP1 = C + 1
N_CORES = 8
NT = T // 128  # 16
EPS = 1e-5
UNITS = [(u // H, u % H) for u in range(2 * H)]  # 12 real units
CORE_UNITS = {0: [0, 1], 1: [2, 3], 2: [4, 5], 3: [6, 7], 4: [8], 5: [9], 6: [10], 7: [11]}
UNIT_SLOT = {}
for _c, _us in CORE_UNITS.items():
    for _s, _u in enumerate(_us):
        UNIT_SLOT[_u] = (_c, _s)

_COMPILED = {}


def build_program():
    nc = bacc.Bacc(trn_type="TRN2", num_devices=N_CORES)

    def _mm(out, lhsT, rhs, start, stop):
        nc.tensor.matmul(out, lhsT, rhs, start=start, stop=stop)

    _mmb = _mm

    def din(name, shape, dt=F32):
        return nc.dram_tensor(name, list(shape), dt, kind="ExternalInput")

    xT_d = din("xT", (C, T), F32R)
    wqk_d = din("wqk", (2, 3, 128, 128), F32R)
    wv_d = din("wv", (3, 128, 128), F32R)
    r1qk_d = din("r1qk", (1, 512), F32R)
    r1v_d = din("r1v", (1, 256), F32R)
    c1qk_d = din("c1qk", (128, 2))
    c1v_d = din("c1v", (128, 1))
    ident_d = din("ident", (128, 128))
    onesc_d = din("onesc", (128, 1), F32R)
    onesr_d = din("onesr", (1, 128), F32R)
    tcol_d = din("tcol", (128, 1))
    sbias_d = din("sbias", (1, 2))
    epsc_d = din("epsc", (128, 1))
    cbias_d = din("cbias", (128, NT))
    wproj_d = din("wproj", (128, H * 3 * 128), BF16)
    bproj_d = din("bproj", (128, 3))
    wf_d = din("wf", (128, 12 * 3 * 128), BF16)
    nwft_d = din("nwft", (1, 1536), BF16)
    ns2f_d = din("ns2f", (1, 1536), BF16)
    c2b_d = din("c2b", (128, 12))
    wf2_d = din("wf2", (128, 3 * 12 * 128), BF16)
    bfc2_d = din("bfc2", (128, 3))
    out_d = nc.dram_tensor("oT", [C, 512], F32, kind="ExternalOutput")

    with TileContext(nc) as tc, nc.allow_low_precision(reason="f32r-typed intermediates (same bits as f32)"):
        with (
            tc.tile_pool(name="const", bufs=1) as cpool,
            tc.tile_pool(name="dram", bufs=1, space="DRAM") as dpool,
            tc.tile_pool(name="ps_wide", bufs=2, space="PSUM") as ppw,
            tc.tile_pool(name="ps_mm", bufs=3, space="PSUM") as ppm,
            tc.tile_pool(name="ps_tr", bufs=1, space="PSUM") as ppt,
            tc.tile_pool(name="ps_trb", bufs=2, space="PSUM") as ppb,
            tc.tile_pool(name="vec", bufs=1) as vp,
            tc.tile_pool(name="qk", bufs=1) as qkp,
            tc.tile_pool(name="tailw", bufs=1) as twp,
        ):
            a2a_in = [dpool.tile([8, 64, 512], BF16, name=f"a2a_in{s}") for s in range(2)]
            a2a_out = [dpool.tile([8, 64, 512], BF16, name=f"a2a_out{s}") for s in range(2)]
            bounce = dpool.tile([2, T], BF16, name="bounce")
            bnc_pview = [bounce[s:s + 1, :].rearrange("a (f p) -> (a p) f", p=128)
                         for s in range(2)]

            ident = cpool.tile([128, 128], F32, tag="ident", name="ident")
            onesc = cpool.tile([128, 1], F32R, tag="onesc", name="onesc")
            onesr = cpool.tile([1, 128], F32R, tag="onesr", name="onesr")
            tcol = cpool.tile([128, 1], F32, tag="tcol", name="tcol")
            sbias = cpool.tile([1, 2], F32, tag="sbias", name="sbias")
            epsc = cpool.tile([128, 1], F32, tag="epsc", name="epsc")
            nc.sync.dma_start(out=ident[:, :], in_=ident_d[:, :])
            nc.sync.dma_start(out=onesc[:, :], in_=onesc_d[:, :])
            nc.sync.dma_start(out=onesr[:, :], in_=onesr_d[:, :])
            nc.sync.dma_start(out=tcol[:, :], in_=tcol_d[:, :])
            nc.sync.dma_start(out=sbias[:, :], in_=sbias_d[:, :])
            nc.sync.dma_start(out=epsc[:, :], in_=epsc_d[:, :])
            identr = cpool.tile([128, 128], F32R, tag="identr", name="identr")
            nc.scalar.copy(identr[:, :], ident[:, :])
            onescf = cpool.tile([128, 1], F32, tag="onescf", name="onescf")
            onesrf = cpool.tile([1, 128], F32, tag="onesrf", name="onesrf")
            nc.scalar.copy(onescf[:, :], onesc[:, :])
            nc.scalar.copy(onesrf[:, :], onesr[:, :])
            identb = cpool.tile([128, 128], BF16, tag="identb", name="identb")
            nc.scalar.copy(identb[:, :], ident[:, :])
            onescb = cpool.tile([128, 1], BF16, tag="onescb", name="onescb")
            nc.scalar.copy(onescb[:, :], onesc[:, :])
            onesrb = cpool.tile([1, 128], BF16, tag="onesrb", name="onesrb")
            nc.scalar.copy(onesrb[:, :], onesr[:, :])

            # persistent per-slot activations (base-partition-0 tiles)
            qT = [qkp.tile([64, T], BF16, tag=f"qT{s}", name=f"qT{s}") for s in range(2)]
            kT = [qkp.tile([64, T], BF16, tag=f"kT{s}", name=f"kT{s}") for s in range(2)]
            vrow = [qkp.tile([128, NT * 64], BF16, tag=f"vrow{s}", name=f"vrow{s}") for s in range(2)]

            # ---------------- phase 1+2: stats + QKV (xt-scoped) ----------------
            with tc.tile_pool(name="xt", bufs=1) as xp:
                xT = [xp.tile([128, T], F32R, tag=f"xt{kc}", name=f"xt{kc}") for kc in range(3)]
                for kc in range(3):
                    nc.sync.dma_start(out=xT[kc][:, :], in_=xT_d[kc * 128:(kc + 1) * 128, :])
                wqk = [[xp.tile([128, 128], F32R, tag=f"wqk{s}{kc}", name=f"wqk{s}{kc}") for kc in range(3)] for s in range(2)]
                wv = [xp.tile([128, 128], F32R, tag=f"wv{kc}", name=f"wv{kc}") for kc in range(3)]
                r1qk = xp.tile([1, 512], F32R, tag="r1qk", name="r1qk")
                r1v = xp.tile([1, 256], F32R, tag="r1v", name="r1v")
                c1qk = xp.tile([128, 2], F32, tag="c1qk", name="c1qk")
                c1v = xp.tile([128, 1], F32, tag="c1v", name="c1v")
                for s in range(2):
                    for kc in range(3):
                        nc.sync.dma_start(out=wqk[s][kc][:, :], in_=wqk_d[s, kc, :, :])
                for kc in range(3):
                    nc.sync.dma_start(out=wv[kc][:, :], in_=wv_d[kc, :, :])
                nc.sync.dma_start(out=r1qk[:, :], in_=r1qk_d[:, :])
                nc.sync.dma_start(out=r1v[:, :], in_=r1v_d[:, :])
                nc.sync.dma_start(out=c1qk[:, :], in_=c1qk_d[:, :])
                nc.sync.dma_start(out=c1v[:, :], in_=c1v_d[:, :])

                # tail weights (bf16): single mega-tile DMAs, hide under attention
                wproj_a = twp.tile([128, H * 3 * 128], BF16, tag="wproj_a", name="wproj_a")
                wf_a = twp.tile([128, 12 * 3 * 128], BF16, tag="wf_a", name="wf_a")
                wf2_a = twp.tile([128, 3 * 12 * 128], BF16, tag="wf2_a", name="wf2_a")
                bproj = twp.tile([128, 3], F32, tag="bproj", name="bproj")
                nwft = twp.tile([1, 1536], BF16, tag="nwft", name="nwft")
                ns2f = twp.tile([1, 1536], BF16, tag="ns2f", name="ns2f")
                c2b = twp.tile([128, 12], F32, tag="c2b", name="c2b")
                bfc2 = twp.tile([128, 3], F32, tag="bfc2", name="bfc2")
                nc.sync.dma_start(out=wproj_a[:, :], in_=wproj_d[:, :])
                nc.sync.dma_start(out=wf_a[:, :], in_=wf_d[:, :])
                nc.sync.dma_start(out=wf2_a[:, :], in_=wf2_d[:, :])
                wproj = [[wproj_a[:, (h * 3 + ec) * 128:(h * 3 + ec + 1) * 128]
                          for ec in range(3)] for h in range(H)]
                wf = [[wf_a[:, (jc * 3 + kc) * 128:(jc * 3 + kc + 1) * 128]
                       for kc in range(3)] for jc in range(12)]
                wf2 = [[wf2_a[:, (ec * 12 + kc) * 128:(ec * 12 + kc + 1) * 128]
                        for kc in range(12)] for ec in range(3)]
                nc.sync.dma_start(out=bproj[:, :], in_=bproj_d[:, :])
                nc.sync.dma_start(out=nwft[:, :], in_=nwft_d[:, :])
                nc.sync.dma_start(out=ns2f[:, :], in_=ns2f_d[:, :])
                nc.sync.dma_start(out=c2b[:, :], in_=c2b_d[:, :])
                nc.sync.dma_start(out=bfc2[:, :], in_=bfc2_d[:, :])

                # ---- stats ----
                mu_row = xp.tile([1, T], F32R, tag="mu_row", name="mu_row")
                for c4 in range(4):
                    wide = ppw.tile([64, 512], F32, tag="wide", name="wide")
                    for kc in range(3):
                        _mm(wide[0:1, :], onesc[:, :],
                            xT[kc][:, c4 * 512:(c4 + 1) * 512], start=(kc == 0), stop=(kc == 2))
                    nc.scalar.activation(mu_row[0:1, c4 * 512:(c4 + 1) * 512],
                                         wide[0:1, :],
                                         AF.Identity, bias=sbias[0:1, 0:1], scale=1.0 / CP1)
                msq_row = xp.tile([1, T], F32, tag="msq_row", name="msq_row")
                for c4 in range(4):
                    ps = ppm.tile([1, 512], F32, tag="mm", name="mm")
                    for kc in range(3):
                        sq = xp.tile([128, 512], F32R, tag="scr", name="scr")
                        nc.scalar.square(sq[:, :], xT[kc][:, c4 * 512:(c4 + 1) * 512])
                        _mm(ps[0:1, :], onesc[:, :], sq[:, :], start=(kc == 0), stop=(kc == 2))
                    nc.scalar.activation(msq_row[0:1, c4 * 512:(c4 + 1) * 512], ps[0:1, :],
                                         AF.Identity, bias=sbias[0:1, 1:2], scale=1.0 / CP1)

                var_row = xp.tile([1, T], F32R, tag="var_row", name="var_row")
                nc.vector.tensor_tensor(var_row[0:1, :], mu_row[0:1, :], mu_row[0:1, :], ALU.mult)
                nc.vector.tensor_tensor(var_row[0:1, :], msq_row[0:1, :], var_row[0:1, :], ALU.subtract)
                # lnv = Ln(var+eps); rstd = Exp(-0.5*lnv) — all scalar, no DVE recip
                nc.scalar.activation(var_row[0:1, :], var_row[0:1, :], AF.Ln, bias=epsc[0:1, 0:1])
                bneg_row = xp.tile([1, T], F32R, tag="bneg_row", name="bneg_row")
                nc.vector.tensor_scalar(bneg_row[0:1, :], mu_row[0:1, :], tcol[0:1, 0:1],
                                        None, ALU.subtract)

                rstd_bc = xp.tile([128, T], F32, tag="rstd_bc", name="rstd_bc")
                for c4 in range(4):
                    ps = ppm.tile([128, 512], F32, tag="mm", name="mm")
                    _mm(ps[:, :], onesr[:, :], var_row[0:1, c4 * 512:(c4 + 1) * 512],
                        start=True, stop=True)
                    nc.scalar.activation(rstd_bc[:, c4 * 512:(c4 + 1) * 512], ps[:, :],
                                         AF.Exp, scale=-0.5)

                # ---- QKV matmuls -> combined (128, T) tiles (xt-scoped) ----
                qk_c = [xp.tile([128, T], BF16, tag=f"qk_c{s}", name=f"qk_c{s}") for s in range(2)]
                v_c = xp.tile([128, T], BF16, tag="v_c", name="v_c")

                def qkv_mat(dst, lhsT_chunks, r1_trow, r1_s1, c1col):
                    for c4 in range(4):
                        sl = slice(c4 * 512, (c4 + 1) * 512)
                        ps = ppm.tile([128, 512], F32, tag="mm", name="mm")
                        for kc in range(3):
                            _mm(ps[:, :], lhsT_chunks[kc][:, :], xT[kc][:, sl],
                                start=(kc == 0), stop=False)
                        _mm(ps[:, :], r1_trow, bneg_row[0:1, sl], start=False, stop=False)
                        _mm(ps[:, :], r1_s1, mu_row[0:1, sl], start=False, stop=True)
                        tmp = xp.tile([128, 512], F32, tag=f"qtmp{c4 % 2}", name=f"qtmp{c4 % 2}")
                        nc.vector.tensor_tensor(tmp[:, :], ps[:, :], rstd_bc[:, sl], ALU.mult)
                        nc.scalar.activation(dst[:, sl], tmp[:, :], AF.Identity,
                                             bias=c1col, scale=1.0)

                for s in range(2):
                    qkv_mat(qk_c[s], wqk[s], r1qk[0:1, (2 * s) * 128:(2 * s) * 128 + 128],
                            r1qk[0:1, (2 * s + 1) * 128:(2 * s + 1) * 128 + 128], c1qk[:, s:s + 1])
                qkv_mat(v_c, wv, r1v[0:1, 0:128], r1v[0:1, 128:256], c1v[:, 0:1])

                # extract base-0 copies (all bf16 now, so plain DMAs off-engine)
                vA = xp.tile([64, T], BF16, tag="vA", name="vA")
                vB = xp.tile([64, T], BF16, tag="vB", name="vB")
                for s in range(2):
                    nc.gpsimd.dma_start(out=qT[s][:, :], in_=qk_c[s][0:64, :])
                    nc.gpsimd.dma_start(out=kT[s][:, :], in_=qk_c[s][64:128, :])
                nc.gpsimd.dma_start(out=vA[:, :], in_=v_c[0:64, :])
                nc.gpsimd.dma_start(out=vB[:, :], in_=v_c[64:128, :])
                # v row-major tiles: vrow[s][:, jt*64:(jt+1)*64] = v[jt-chunk].T
                for s, vsrc in ((0, vA), (1, vB)):
                    for g0 in range(0, NT, 4):
                        tr = ppb.tile([128, 512], BF16, tag="trb", name="trb")
                        for gi in range(4):
                            jt = g0 + gi
                            nc.tensor.transpose(tr[:, gi * 128:gi * 128 + 64],
                                                vsrc[:, jt * 128:(jt + 1) * 128], identb[0:64, 0:64])
                        for gi in range(4):
                            nc.scalar.copy(vrow[s][:, (g0 + gi) * 64:(g0 + gi + 1) * 64],
                                           tr[:, gi * 128:gi * 128 + 64])

            # ---------------- phase 3: attention per slot ----------------
            # 1-round sinkhorn (matches 6-iter reference to ~2e-6):
            #   alpha = 1/(rowsum exp(c)) = 1/(sacc + (T-L))   [T*a, from exp accum]
            #   b = 1/(S'^T alpha + sum(alpha))                [one matvec pass]
            #   y^T = alpha ∘ (S'(b∘V) + colsum(b∘V))
            with (
                tc.tile_pool(name="sp", bufs=1) as spp,
                tc.tile_pool(name="spt", bufs=1) as sptp,
                tc.tile_pool(name="att_misc", bufs=1) as amp,
            ):
                cbias = amp.tile([128, NT], F32, tag="cbias", name="cbias")
                nc.sync.dma_start(out=cbias[:, :], in_=cbias_d[:, :])
                sp_s, spt_s, zall_s = {}, {}, {}
                # pass 1: QK + E + z for both slots (keeps PE fed while scalar exps)
                for s in range(2):
                    sp = [spp.tile([128, (it + 1) * 128], BF16, tag=f"sp{s}{it}", name=f"sp{s}{it}") for it in range(NT)]
                    spt = [sptp.tile([128, (NT - jt) * 128], BF16, tag=f"spt{s}{jt}", name=f"spt{s}{jt}") for jt in range(NT)]
                    e = [spt[NT - 1 - it] for it in range(NT)]  # aliases (same size, bf16)
                    sp_s[s], spt_s[s] = sp, spt

                    zall = amp.tile([128, NT], F32, tag=f"zall{s}", name=f"zall{s}")
                    zall_s[s] = zall
                    for it in range(NT):
                        L = (it + 1) * 128
                        d0 = it * 128
                        nch = (L + 511) // 512
                        for c4 in range(nch):
                            lo, hi = c4 * 512, min(L, (c4 + 1) * 512)
                            ps = ppm.tile([128, 512], F32, tag="mm", name="mm")
                            _mm(ps[:, 0:hi - lo], qT[s][:, d0:d0 + 128], kT[s][:, lo:hi],
                                start=True, stop=True)
                            nc.scalar.activation(e[it][:, lo:hi], ps[:, 0:hi - lo],
                                                 AF.Exp, scale=0.125)
                        nc.gpsimd.affine_select(out=e[it][:, d0:L], in_=e[it][:, d0:L],
                                                compare_op=ALU.is_ge, fill=0.0, base=0,
                                                pattern=[[-1, 128]], channel_multiplier=1)
                        nc.vector.tensor_reduce(zall[:, it:it + 1], e[it][:, 0:L],
                                                axis=AXX, op=ALU.add)

                # pass 2: per-slot sinkhorn tail; slot-0's AllToAll hides under slot 1
                for s in range(2):
                    sp, spt, zall = sp_s[s], spt_s[s], zall_s[s]
                    e = [spt[NT - 1 - it] for it in range(NT)]
                    rz = amp.tile([128, NT], F32, tag=f"rz{s}", name=f"rz{s}")
                    nc.vector.reciprocal(rz[:, :], zall[:, :])

                    # sp = exp(rz*E) - 1; accum gives rowsum(exp(c)) over stored cols
                    sacc = amp.tile([128, NT], F32, tag=f"sacc{s}", name=f"sacc{s}")
                    for it in range(NT):
                        L = (it + 1) * 128
                        nc.scalar.activation(sp[it][:, :], e[it][:, 0:L], AF.Exp,
                                             scale=rz[:, it:it + 1],
                                             accum_out=sacc[:, it:it + 1])
                        nc.vector.tensor_scalar(sp[it][:, :], sp[it][:, :], -1.0, None, ALU.add)

                    # alpha = 1/(sacc + (T - L)); bounce to row form (off critical path)
                    alpha = amp.tile([128, NT], F32R, tag=f"alpha{s}", name=f"alpha{s}")
                    nc.vector.tensor_tensor(alpha[:, :], sacc[:, :], cbias[:, :], ALU.add)
                    nc.vector.reciprocal(alpha[:, :], alpha[:, :])
                    al16 = amp.tile([128, NT], BF16, tag=f"al16{s}", name=f"al16{s}")
                    nc.vector.tensor_copy(al16[:, :], alpha[:, :])
                    nc.sync.dma_start(out=bnc_pview[s], in_=al16[:, :])
                    arow = amp.tile([1, T], BF16, tag="arow", name="arow")
                    nc.sync.dma_start(out=arow[0:1, :], in_=bounce[s:s + 1, :])

                    # transposes: sp (bf16) -> spt (bf16); copies 1/3 scalar, 2/3 vector
                    ncopy = 0
                    for jt in range(NT):
                        nit = NT - jt
                        for g0 in range(0, nit, 4):
                            gn = min(4, nit - g0)
                            tr = ppb.tile([128, 512], BF16, tag="trb", name="trb")
                            for gi in range(gn):
                                it = jt + g0 + gi
                                nc.tensor.transpose(tr[:, gi * 128:(gi + 1) * 128],
                                                    sp[it][:, jt * 128:(jt + 1) * 128],
                                                    identb[:, :])
                            if ncopy % 3 == 0:
                                nc.scalar.copy(spt[jt][:, g0 * 128:(g0 + gn) * 128], tr[:, 0:gn * 128])
                            else:
                                nc.vector.tensor_copy(spt[jt][:, g0 * 128:(g0 + gn) * 128], tr[:, 0:gn * 128])
                            ncopy += 1

                    # ---- one matvec pass: r = S'^T alpha (row form), bank-outer ----
                    ared = amp.tile([128, 1], F32, tag=f"ared{s}", name=f"ared{s}")
                    nc.vector.tensor_reduce(ared[:, :], alpha[:, :], axis=AXX, op=ALU.add)
                    ps1 = ppm.tile([1, 512], F32, tag="mm", name="mm")
                    _mm(ps1[0:1, 0:1], onescf[:, :], ared[:, :], start=True, stop=True)
                    asum = amp.tile([1, 1], F32, tag=f"asum{s}", name=f"asum{s}")
                    nc.scalar.copy(asum[0:1, :], ps1[0:1, 0:1])
                    brow = amp.tile([1, T], F32, tag="brow", name="brow")
                    bps = ppt.tile([128, 512], F32, tag="tr", name="tr")
                    for c4 in range(4):
                        lo, hi = c4 * 512, (c4 + 1) * 512
                        wps = ppw.tile([64, 512], F32, tag="wide", name="wide")
                        for it in range(4 * c4, NT):
                            L = (it + 1) * 128
                            shi = min(L, hi) - lo
                            _mm(wps[0:1, 0:shi], al16[:, it:it + 1], sp[it][:, lo:lo + shi],
                                start=(it == c4 * 4), stop=(it == NT - 1))
                        # brow holds r + sum(alpha); reciprocal happens in column space
                        nc.scalar.activation(brow[0:1, lo:hi], wps[0:1, :], AF.Identity,
                                             bias=asum[0:1, 0:1], scale=1.0)
                        for jq in range(4):
                            jt = 4 * c4 + jq
                            nc.tensor.transpose(bps[:, jt:jt + 1],
                                                brow[0:1, jt * 128:(jt + 1) * 128],
                                                ident[0:1, 0:1])
                    bcol = amp.tile([128, NT], F32, tag=f"bcol{s}", name=f"bcol{s}")
                    nc.scalar.copy(bcol[:, :], bps[:, 0:NT])
                    nc.vector.reciprocal(bcol[:, :], bcol[:, :])
                    bv = []
                    for jt in range(NT):
                        bvt = amp.tile([128, 64], BF16, tag=f"bv{jt}", name=f"bv{jt}")
                        nc.vector.tensor_scalar(bvt[:, :], vrow[s][:, jt * 64:(jt + 1) * 64],
                                                bcol[:, jt:jt + 1], None, ALU.mult)
                        bv.append(bvt)

                    # ---- y^T = alpha ∘ (S' @ (b∘V) + colsum(b∘V)), bank-outer ----
                    wcps = ppt.tile([128, 512], F32, tag="tr", name="tr")
                    for jt in range(NT):
                        _mm(wcps[0:1, 0:64], onescb[:, :], bv[jt][:, :],
                            start=(jt == 0), stop=(jt == NT - 1))
                    wrow = amp.tile([1, 64], F32, tag=f"wrow{s}", name=f"wrow{s}")
                    nc.scalar.copy(wrow[0:1, :], wcps[0:1, 0:64])
                    wtp = ppt.tile([128, 512], F32, tag="tr", name="tr")
                    nc.tensor.transpose(wtp[0:64, 0:1], wrow[0:1, :], ident[0:1, 0:1])
                    tw = amp.tile([64, 1], F32, tag=f"tw{s}", name=f"tw{s}")
                    nc.scalar.copy(tw[:, :], wtp[0:64, 0:1])
                    for c4 in range(4):
                        lo, hi = c4 * 512, (c4 + 1) * 512
                        sl = slice(lo, hi)
                        yps = ppw.tile([64, 512], F32, tag="wide", name="wide")
                        for jt in range(0, min(NT, 4 * c4 + 4)):
                            j0 = jt * 128
                            slo = max(lo, j0)
                            _mmb(yps[:, slo - lo:512], bv[jt][:, :],
                                 spt[jt][:, slo - j0:hi - j0],
                                 start=(jt == 0), stop=(jt == min(NT - 1, 4 * c4 + 3)))
                        psa = ppm.tile([128, 512], F32, tag="mm", name="mm")
                        _mm(psa[0:64, :], onesrb[0:1, 0:64], arow[0:1, sl], start=True, stop=True)
                        abc = amp.tile([64, 512], F32R, tag="abc", name="abc")
                        nc.scalar.copy(abc[:, :], psa[0:64, :])
                        ytmp = amp.tile([64, 512], BF16, tag="ytmp", name="ytmp")
                        nc.scalar.activation(ytmp[:, :], yps[:, :], AF.Identity,
                                             bias=tw[:, 0:1], scale=1.0)
                        nc.vector.tensor_tensor(ytmp[:, :], ytmp[:, :], abc[:, :], ALU.mult)
                        for grp in range(2):
                            nc.gpsimd.dma_start(out=a2a_in[s][grp * 4 + c4, :, :],
                                                in_=ytmp[:, :])

                    # per-slot AllToAll: slot 0's overlaps slot 1's compute
                    nc.gpsimd.collective_compute(
                        "AllToAll", ALU.bypass,
                        replica_groups=[list(range(N_CORES))],
                        ins=[a2a_in[s].opt()],
                        outs=[a2a_out[s].opt()],
                    )

            # ---------------- phase 5: proj + LN2 + MLP ----------------
            with tc.tile_pool(name="tail", bufs=1) as tp:
                stk0 = [tp.tile([128, 512], BF16, tag=f"stk0{h}", name=f"stk0{h}") for h in range(H)]
                for h in range(H):
                    c0, s0 = UNIT_SLOT[h]
                    c1_, s1_ = UNIT_SLOT[H + h]
                    nc.gpsimd.dma_start(out=stk0[h][0:64, :], in_=a2a_out[s0][c0, :, :])
                    nc.gpsimd.dma_start(out=stk0[h][64:128, :], in_=a2a_out[s1_][c1_, :, :])

                hT = [tp.tile([128, 512], F32R, tag=f"ht{ec}", name=f"ht{ec}") for ec in range(3)]
                for ec in range(3):
                    ps = ppm.tile([128, 512], F32, tag="mm", name="mm")
                    for h in range(H):
                        _mm(ps[:, :], wproj[h][ec], stk0[h][:, :],
                            start=(h == 0), stop=(h == H - 1))
                    nc.scalar.activation(hT[ec][:, :], ps[:, :], AF.Identity,
                                         bias=bproj[:, ec:ec + 1], scale=1.0)

                mu2ps = ppm.tile([1, 512], F32, tag="mm", name="mm")
                for ec in range(3):
                    _mm(mu2ps[0:1, :], onesc[:, :], hT[ec][:, :], start=(ec == 0), stop=(ec == 2))
                mu2r = tp.tile([1, 512], F32R, tag="mu2r", name="mu2r")
                nc.scalar.activation(mu2r[0:1, :], mu2ps[0:1, :], AF.Identity,
                                     bias=sbias[0:1, 0:1], scale=1.0 / CP1)
                scr2 = tp.tile([128, 512], F32R, tag="scr2", name="scr2")
                msq2ps = ppm.tile([1, 512], F32, tag="mm", name="mm")
                for ec in range(3):
                    nc.scalar.square(scr2[:, :], hT[ec][:, :])
                    _mm(msq2ps[0:1, :], onesc[:, :], scr2[:, :], start=(ec == 0), stop=(ec == 2))
                msq2r = tp.tile([1, 512], F32, tag="msq2r", name="msq2r")
                nc.scalar.activation(msq2r[0:1, :], msq2ps[0:1, :], AF.Identity,
                                     bias=sbias[0:1, 1:2], scale=1.0 / CP1)
                v2r = tp.tile([1, 512], F32R, tag="v2r", name="v2r")
                nc.vector.tensor_tensor(v2r[0:1, :], mu2r[0:1, :], mu2r[0:1, :], ALU.mult)
                nc.vector.tensor_tensor(v2r[0:1, :], msq2r[0:1, :], v2r[0:1, :], ALU.subtract)
                nc.scalar.activation(v2r[0:1, :], v2r[0:1, :], AF.Ln, bias=epsc[0:1, 0:1])
                # rstd2 = Exp(-0.5*ln(var+eps)) straight from the broadcast psum
                ps = ppm.tile([128, 512], F32, tag="mm", name="mm")
                _mm(ps[:, :], onesr[:, :], v2r[0:1, :], start=True, stop=True)
                rstd2bc = tp.tile([128, 512], F32, tag="rstd2bc", name="rstd2bc")
                nc.scalar.activation(rstd2bc[:, :], ps[:, :], AF.Exp, scale=-0.5)
                rstd2r = tp.tile([1, 512], F32R, tag="rstd2r", name="rstd2r")
                nc.vector.tensor_copy(rstd2r[0:1, :], rstd2bc[0:1, :])
                m2rr = tp.tile([1, 512], BF16, tag="m2rr", name="m2rr")
                b2rr = tp.tile([1, 512], BF16, tag="b2rr", name="b2rr")
                nc.vector.tensor_tensor(m2rr[0:1, :], mu2r[0:1, :], rstd2r[0:1, :], ALU.mult)
                b2f = tp.tile([1, 512], F32R, tag="b2f", name="b2f")
                nc.vector.tensor_scalar(b2f[0:1, :], mu2r[0:1, :], tcol[0:1, 0:1], None, ALU.subtract)
                nc.vector.tensor_tensor(b2rr[0:1, :], b2f[0:1, :], rstd2r[0:1, :], ALU.mult)
                hs = [tp.tile([128, 512], BF16, tag=f"hs{ec}", name=f"hs{ec}") for ec in range(3)]
                for ec in range(3):
                    nc.vector.tensor_tensor(hs[ec][:, :], hT[ec][:, :], rstd2bc[:, :], ALU.mult)

                mT = [tp.tile([128, 512], BF16, tag=f"mt{jc}", name=f"mt{jc}") for jc in range(12)]
                for jc in range(12):
                    ps = ppm.tile([128, 512], F32, tag="mm", name="mm")
                    for kc in range(3):
                        _mm(ps[:, :], wf[jc][kc], hs[kc][:, :], start=(kc == 0), stop=False)
                    _mm(ps[:, :], ns2f[0:1, jc * 128:(jc + 1) * 128], m2rr[0:1, :], start=False, stop=False)
                    _mm(ps[:, :], nwft[0:1, jc * 128:(jc + 1) * 128], b2rr[0:1, :], start=False, stop=True)
                    nc.scalar.activation(mT[jc][:, :], ps[:, :], AF.Gelu,
                                         bias=c2b[:, jc:jc + 1], scale=1.0)
                for ec in range(3):
                    ps = ppm.tile([128, 512], F32, tag="mm", name="mm")
                    for kc in range(12):
                        _mm(ps[:, :], wf2[ec][kc], mT[kc][:, :],
                            start=(kc == 0), stop=(kc == 11))
                    oT = tp.tile([128, 512], F32, tag=f"ot{ec}", name=f"ot{ec}")
                    nc.scalar.activation(oT[:, :], ps[:, :], AF.Identity,
                                         bias=bfc2[:, ec:ec + 1], scale=1.0)
                    nc.sync.dma_start(out=out_d[ec * 128:(ec + 1) * 128, :], in_=oT[:, :])

    nc.compile()
    return nc


def host_prep(inputs):
    x = np.asarray(inputs["x"], np.float32)
    t = float(np.asarray(inputs["t"]).reshape(-1)[0])
    w1 = np.asarray(inputs["ln1_w"], np.float32); b1 = np.asarray(inputs["ln1_b"], np.float32)
    Wa = np.asarray(inputs["attn_w"], np.float32); ba = np.asarray(inputs["attn_b"], np.float32)
    Wp_ = w1[:, None] * Wa
    c1 = b1 @ Wa + ba
    Wa_main, Wa_trow = Wp_[:C], Wp_[C]
    s1 = Wp_[:C].sum(axis=0)
    w2 = np.asarray(inputs["ln2_w"], np.float32); b2 = np.asarray(inputs["ln2_b"], np.float32)
    Wf = np.asarray(inputs["fc_w"], np.float32); bf = np.asarray(inputs["fc_b"], np.float32)
    Wf_p = w2[:, None] * Wf
    c2 = b2 @ Wf + bf
    Wf_main, Wf_trow = Wf_p[:C], Wf_p[C]
    s2f = Wf_p[:C].sum(axis=0)
    Wpj = np.asarray(inputs["proj_w"], np.float32); bpj = np.asarray(inputs["proj_b"], np.float32)
    Wf2 = np.asarray(inputs["fc2_w"], np.float32); bf2 = np.asarray(inputs["fc2_b"], np.float32)

    common = {
        "ident": np.eye(128, dtype=np.float32),
        "onesc": np.ones((128, 1), np.float32),
        "onesr": np.ones((1, 128), np.float32),
        "tcol": np.full((128, 1), t, np.float32),
        "sbias": np.array([[t / CP1, t * t / CP1]], np.float32),
        "epsc": np.full((128, 1), EPS, np.float32),
        "cbias": np.broadcast_to(
            np.array([T - (it + 1) * 128 for it in range(NT)], np.float32),
            (128, NT)).copy(),
        "bproj": bpj.reshape(3, 128).T.astype(np.float32).copy(),
        "c2b": c2.reshape(12, 128).T.astype(np.float32).copy(),
        "bfc2": bf2.reshape(3, 128).T.astype(np.float32).copy(),
        "nwft": (-Wf_trow)[None, :].astype(BF16NP).copy(),
        "ns2f": (-s2f)[None, :].astype(BF16NP).copy(),
        "wf": np.stack([np.stack([Wf_main[kc * 128:(kc + 1) * 128, jc * 128:(jc + 1) * 128]
                                  for kc in range(3)]) for jc in range(12)])
              .transpose(2, 0, 1, 3).reshape(128, -1).astype(BF16NP).copy(),
        "wf2": np.stack([np.stack([Wf2[kc * 128:(kc + 1) * 128, ec * 128:(ec + 1) * 128]
                                   for kc in range(12)]) for ec in range(3)])
               .transpose(2, 0, 1, 3).reshape(128, -1).astype(BF16NP).copy(),
    }

    in_maps = []
    for c in range(N_CORES):
        units = CORE_UNITS[c]
        myb = UNITS[units[0]][0]
        m = dict(common)
        m["xT"] = np.ascontiguousarray(x[myb].T)
        shard_b = c // 4  # batch of the row shard this core finishes (receiver side)
        wproj = np.zeros((H, 3, 128, 128), np.float32)
        for h in range(H):
            for ec in range(3):
                blk = Wpj[h * HD:(h + 1) * HD, ec * 128:(ec + 1) * 128]
                if shard_b == 0:
                    wproj[h, ec, 0:64] = blk
                else:
                    wproj[h, ec, 64:128] = blk
        m["wproj"] = wproj.transpose(2, 0, 1, 3).reshape(128, -1).astype(BF16NP).copy()
        wqk = np.zeros((2, 3, 128, 128), np.float32)
        r1qk = np.zeros((1, 512), np.float32)
        c1qk = np.zeros((128, 2), np.float32)
        wv = np.zeros((3, 128, 128), np.float32)
        r1v = np.zeros((1, 256), np.float32)
        c1v = np.zeros((128, 1), np.float32)
        for s, u in enumerate(units):
            _, h = UNITS[u]
            cq = slice(h * HD, (h + 1) * HD)
            ck = slice(C + h * HD, C + (h + 1) * HD)
            cv = slice(2 * C + h * HD, 2 * C + (h + 1) * HD)
            for kc in range(3):
                wqk[s, kc, :, 0:64] = Wa_main[kc * 128:(kc + 1) * 128, cq]
                wqk[s, kc, :, 64:128] = Wa_main[kc * 128:(kc + 1) * 128, ck]
                wv[kc, :, s * 64:(s + 1) * 64] = Wa_main[kc * 128:(kc + 1) * 128, cv]
            base = 2 * s * 128
            r1qk[0, base:base + 64] = -Wa_trow[cq]; r1qk[0, base + 64:base + 128] = -Wa_trow[ck]
            r1qk[0, base + 128:base + 192] = -s1[cq]; r1qk[0, base + 192:base + 256] = -s1[ck]
            r1v[0, s * 64:(s + 1) * 64] = -Wa_trow[cv]
            r1v[0, 128 + s * 64:128 + (s + 1) * 64] = -s1[cv]
            c1qk[0:64, s] = c1[cq]; c1qk[64:128, s] = c1[ck]
            c1v[s * 64:(s + 1) * 64, 0] = c1[cv]
        m["wqk"] = wqk; m["r1qk"] = r1qk; m["c1qk"] = c1qk
        m["wv"] = wv; m["r1v"] = r1v; m["c1v"] = c1v
        in_maps.append(m)
    return in_maps


def kernel(**inputs):
    if "nc" not in _COMPILED:
        _COMPILED["nc"] = build_program()
    nc = _COMPILED["nc"]
    in_maps = host_prep(inputs)
    res = run_bass_kernel_spmd(nc, in_maps, list(range(N_CORES)))
    out = np.zeros((B, T, C), np.float32)
    for c in range(N_CORES):
        oT = res.results[c]["oT"]
        b, t0 = c // 4, (c % 4) * 512
        out[b, t0:t0 + 512, :] = oT.T
    return out



# revision 75
# speedup vs baseline: 1.1065x; 1.1065x over previous
"""Trainium2 Bass kernel for nn_Block_87428354277599 (sinkhorn-attention transformer block).

Self-contained: hardcodes shapes/sharding. kernel(**inputs) -> (2, 2048, 384) f32.

Sharding (8 cores, SPMD):
- 12 (batch, head) units padded to 16 slots: every core runs 2 attention slots
  (cores 4-7's slot 1 gets zero weights; its junk output is never consumed).
- LN1/LN2 are folded into the QKV / MLP matmuls via host-precomputed weight folds
  plus rank-1 corrections (mu and t-column terms) accumulated on the PE.
- Sinkhorn on the row-softmaxed causal attention == multiplicative matrix scaling
  of S = exp(P). S-1 is lower-triangular, so only the lower triangle (S' = S-1)
  is stored SBUF-resident in both layouts (S' f32, S'^T bf16); the all-ones part
  of S becomes global-sum corrections (kept f32). All matvecs run on the PE.
- y^T slices are exchanged with one AllToAll (each sender duplicates its slices
  into both batch shard groups; receivers mask the wrong batch via zeroed halves
  of the duplicated proj weights). proj+LN2+MLP run row-sharded (512 rows/core).
"""

import numpy as np
import ml_dtypes

BF16NP = ml_dtypes.bfloat16

import concourse.bacc as bacc
import concourse.mybir as mybir
from concourse.tile import TileContext
from concourse.bass_utils import run_bass_kernel_spmd

F32 = mybir.dt.float32
BF16 = mybir.dt.bfloat16
F32R = mybir.dt.float32r
AF = mybir.ActivationFunctionType
ALU = mybir.AluOpType
AXX = mybir.AxisListType.X

B, T, C, H, HD = 2, 2048, 384, 6, 64
CP1 = C + 1
N_CORES = 8
NT = T // 128  # 16
EPS = 1e-5
UNITS = [(u // H, u % H) for u in range(2 * H)]  # 12 real units
CORE_UNITS = {0: [0, 1], 1: [2, 3], 2: [4, 5], 3: [6, 7], 4: [8], 5: [9], 6: [10], 7: [11]}
UNIT_SLOT = {}
for _c, _us in CORE_UNITS.items():
    for _s, _u in enumerate(_us):
        UNIT_SLOT[_u] = (_c, _s)

_COMPILED = {}


def build_program():
    nc = bacc.Bacc(trn_type="TRN2", num_devices=N_CORES)

    def _mm(out, lhsT, rhs, start, stop):
        nc.tensor.matmul(out, lhsT, rhs, start=start, stop=stop)

    _mmb = _mm

    def din(name, shape, dt=F32):
        return nc.dram_tensor(name, list(shape), dt, kind="ExternalInput")

    xT_d = din("xT", (C, T), F32R)
    wqk_d = din("wqk", (2, 3, 128, 128), F32R)
    wv_d = din("wv", (3, 128, 128), F32R)
    r1qk_d = din("r1qk", (1, 512), F32R)
    r1v_d = din("r1v", (1, 256), F32R)
    c1qk_d = din("c1qk", (128, 2))
    c1v_d = din("c1v", (128, 1))
    ident_d = din("ident", (128, 128))
    onesc_d = din("onesc", (128, 1), F32R)
    onesr_d = din("onesr", (1, 128), F32R)
    tcol_d = din("tcol", (128, 1))
    sbias_d = din("sbias", (1, 2))
    epsc_d = din("epsc", (128, 1))
    cbias_d = din("cbias", (128, NT))
    wproj_d = din("wproj", (128, H * 3 * 128), BF16)
    bproj_d = din("bproj", (128, 3))
    wf_d = din("wf", (128, 12 * 3 * 128), BF16)
    nwft_d = din("nwft", (1, 1536), BF16)
    ns2f_d = din("ns2f", (1, 1536), BF16)
    c2b_d = din("c2b", (128, 12))
    wf2_d = din("wf2", (128, 3 * 12 * 128), BF16)
    bfc2_d = din("bfc2", (128, 3))
    out_d = nc.dram_tensor("oT", [C, 512], F32, kind="ExternalOutput")

    with TileContext(nc) as tc, nc.allow_low_precision(reason="f32r-typed intermediates (same bits as f32)"):
        with (
            tc.tile_pool(name="const", bufs=1) as cpool,
            tc.tile_pool(name="dram", bufs=1, space="DRAM") as dpool,
            tc.tile_pool(name="ps_wide", bufs=2, space="PSUM") as ppw,
            tc.tile_pool(name="ps_mm", bufs=3, space="PSUM") as ppm,
            tc.tile_pool(name="ps_tr", bufs=1, space="PSUM") as ppt,
            tc.tile_pool(name="ps_trb", bufs=2, space="PSUM") as ppb,
            tc.tile_pool(name="vec", bufs=1) as vp,
            tc.tile_pool(name="qk", bufs=1) as qkp,
            tc.tile_pool(name="tailw", bufs=1) as twp,
        ):
            a2a_in = [dpool.tile([8, 64, 512], BF16, name=f"a2a_in{s}") for s in range(2)]
            a2a_out = [dpool.tile([8, 64, 512], BF16, name=f"a2a_out{s}") for s in range(2)]
            bounce = dpool.tile([2, T], BF16, name="bounce")
            bnc_pview = [bounce[s:s + 1, :].rearrange("a (f p) -> (a p) f", p=128)
                         for s in range(2)]

            ident = cpool.tile([128, 128], F32, tag="ident", name="ident")
            onesc = cpool.tile([128, 1], F32R, tag="onesc", name="onesc")
            onesr = cpool.tile([1, 128], F32R, tag="onesr", name="onesr")
            tcol = cpool.tile([128, 1], F32, tag="tcol", name="tcol")
            sbias = cpool.tile([1, 2], F32, tag="sbias", name="sbias")
            epsc = cpool.tile([128, 1], F32, tag="epsc", name="epsc")
            nc.sync.dma_start(out=ident[:, :], in_=ident_d[:, :])
            nc.sync.dma_start(out=onesc[:, :], in_=onesc_d[:, :])
            nc.sync.dma_start(out=onesr[:, :], in_=onesr_d[:, :])
            nc.sync.dma_start(out=tcol[:, :], in_=tcol_d[:, :])
            nc.sync.dma_start(out=sbias[:, :], in_=sbias_d[:, :])
            nc.sync.dma_start(out=epsc[:, :], in_=epsc_d[:, :])
            identr = cpool.tile([128, 128], F32R, tag="identr", name="identr")
            nc.scalar.copy(identr[:, :], ident[:, :])
            onescf = cpool.tile([128, 1], F32, tag="onescf", name="onescf")
            onesrf = cpool.tile([1, 128], F32, tag="onesrf", name="onesrf")
            nc.scalar.copy(onescf[:, :], onesc[:, :])
            nc.scalar.copy(onesrf[:, :], onesr[:, :])
            identb = cpool.tile([128, 128], BF16, tag="identb", name="identb")
            nc.scalar.copy(identb[:, :], ident[:, :])
            onescb = cpool.tile([128, 1], BF16, tag="onescb", name="onescb")
            nc.scalar.copy(onescb[:, :], onesc[:, :])
            onesrb = cpool.tile([1, 128], BF16, tag="onesrb", name="onesrb")
            nc.scalar.copy(onesrb[:, :], onesr[:, :])

            # persistent per-slot activations (base-partition-0 tiles)
            qT = [qkp.tile([64, T], BF16, tag=f"qT{s}", name=f"qT{s}") for s in range(2)]
            kT = [qkp.tile([64, T], BF16, tag=f"kT{s}", name=f"kT{s}") for s in range(2)]
            vrow = [qkp.tile([128, NT * 64], BF16, tag=f"vrow{s}", name=f"vrow{s}") for s in range(2)]

            # ---------------- phase 1+2: stats + QKV (xt-scoped) ----------------
            with tc.tile_pool(name="xt", bufs=1) as xp:
                xT = [xp.tile([128, T], F32R, tag=f"xt{kc}", name=f"xt{kc}") for kc in range(3)]
                for kc in range(3):
                    nc.sync.dma_start(out=xT[kc][:, :], in_=xT_d[kc * 128:(kc + 1) * 128, :])
                wqk = [[xp.tile([128, 128], F32R, tag=f"wqk{s}{kc}", name=f"wqk{s}{kc}") for kc in range(3)] for s in range(2)]
                wv = [xp.tile([128, 128], F32R, tag=f"wv{kc}", name=f"wv{kc}") for kc in range(3)]
                r1qk = xp.tile([1, 512], F32R, tag="r1qk", name="r1qk")
                r1v = xp.tile([1, 256], F32R, tag="r1v", name="r1v")
                c1qk = xp.tile([128, 2], F32, tag="c1qk", name="c1qk")
                c1v = xp.tile([128, 1], F32, tag="c1v", name="c1v")
                for s in range(2):
                    for kc in range(3):
                        nc.sync.dma_start(out=wqk[s][kc][:, :], in_=wqk_d[s, kc, :, :])
                for kc in range(3):
                    nc.sync.dma_start(out=wv[kc][:, :], in_=wv_d[kc, :, :])
                nc.sync.dma_start(out=r1qk[:, :], in_=r1qk_d[:, :])
                nc.sync.dma_start(out=r1v[:, :], in_=r1v_d[:, :])
                nc.sync.dma_start(out=c1qk[:, :], in_=c1qk_d[:, :])
                nc.sync.dma_start(out=c1v[:, :], in_=c1v_d[:, :])

                # tail weights (bf16): single mega-tile DMAs, hide under attention
                wproj_a = twp.tile([128, H * 3 * 128], BF16, tag="wproj_a", name="wproj_a")
                wf_a = twp.tile([128, 12 * 3 * 128], BF16, tag="wf_a", name="wf_a")
                wf2_a = twp.tile([128, 3 * 12 * 128], BF16, tag="wf2_a", name="wf2_a")
                bproj = twp.tile([128, 3], F32, tag="bproj", name="bproj")
                nwft = twp.tile([1, 1536], BF16, tag="nwft", name="nwft")
                ns2f = twp.tile([1, 1536], BF16, tag="ns2f", name="ns2f")
                c2b = twp.tile([128, 12], F32, tag="c2b", name="c2b")
                bfc2 = twp.tile([128, 3], F32, tag="bfc2", name="bfc2")
                nc.sync.dma_start(out=wproj_a[:, :], in_=wproj_d[:, :])
                nc.sync.dma_start(out=wf_a[:, :], in_=wf_d[:, :])
                nc.sync.dma_start(out=wf2_a[:, :], in_=wf2_d[:, :])
                wproj = [[wproj_a[:, (h * 3 + ec) * 128:(h * 3 + ec + 1) * 128]
                          for ec in range(3)] for h in range(H)]
                wf = [[wf_a[:, (jc * 3 + kc) * 128:(jc * 3 + kc + 1) * 128]
                       for kc in range(3)] for jc in range(12)]
                wf2 = [[wf2_a[:, (ec * 12 + kc) * 128:(ec * 12 + kc + 1) * 128]
                        for kc in range(12)] for ec in range(3)]
                nc.sync.dma_start(out=bproj[:, :], in_=bproj_d[:, :])
                nc.sync.dma_start(out=nwft[:, :], in_=nwft_d[:, :])
                nc.sync.dma_start(out=ns2f[:, :], in_=ns2f_d[:, :])
                nc.sync.dma_start(out=c2b[:, :], in_=c2b_d[:, :])
                nc.sync.dma_start(out=bfc2[:, :], in_=bfc2_d[:, :])

                # ---- stats ----
                mu_row = xp.tile([1, T], F32R, tag="mu_row", name="mu_row")
                for c4 in range(4):
                    wide = ppw.tile([64, 512], F32, tag="wide", name="wide")
                    for kc in range(3):
                        _mm(wide[0:1, :], onesc[:, :],
                            xT[kc][:, c4 * 512:(c4 + 1) * 512], start=(kc == 0), stop=(kc == 2))
                    nc.scalar.activation(mu_row[0:1, c4 * 512:(c4 + 1) * 512],
                                         wide[0:1, :],
                                         AF.Identity, bias=sbias[0:1, 0:1], scale=1.0 / CP1)
                msq_row = xp.tile([1, T], F32, tag="msq_row", name="msq_row")
                for c4 in range(4):
                    ps = ppm.tile([1, 512], F32, tag="mm", name="mm")
                    for kc in range(3):
                        sq = xp.tile([128, 512], F32R, tag="scr", name="scr")
                        nc.scalar.square(sq[:, :], xT[kc][:, c4 * 512:(c4 + 1) * 512])
                        _mm(ps[0:1, :], onesc[:, :], sq[:, :], start=(kc == 0), stop=(kc == 2))
                    nc.scalar.activation(msq_row[0:1, c4 * 512:(c4 + 1) * 512], ps[0:1, :],
                                         AF.Identity, bias=sbias[0:1, 1:2], scale=1.0 / CP1)

                var_row = xp.tile([1, T], F32R, tag="var_row", name="var_row")
                nc.vector.tensor_tensor(var_row[0:1, :], mu_row[0:1, :], mu_row[0:1, :], ALU.mult)
                nc.vector.tensor_tensor(var_row[0:1, :], msq_row[0:1, :], var_row[0:1, :], ALU.subtract)
                # lnv = Ln(var+eps); rstd = Exp(-0.5*lnv) — all scalar, no DVE recip
                nc.scalar.activation(var_row[0:1, :], var_row[0:1, :], AF.Ln, bias=epsc[0:1, 0:1])
                bneg_row = xp.tile([1, T], F32R, tag="bneg_row", name="bneg_row")
                nc.vector.tensor_scalar(bneg_row[0:1, :], mu_row[0:1, :], tcol[0:1, 0:1],
                                        None, ALU.subtract)

                rstd_bc = xp.tile([128, T], F32, tag="rstd_bc", name="rstd_bc")
                for c4 in range(4):
                    ps = ppm.tile([128, 512], F32, tag="mm", name="mm")
                    _mm(ps[:, :], onesr[:, :], var_row[0:1, c4 * 512:(c4 + 1) * 512],
                        start=True, stop=True)
                    nc.scalar.activation(rstd_bc[:, c4 * 512:(c4 + 1) * 512], ps[:, :],
                                         AF.Exp, scale=-0.5)

                # ---- QKV matmuls -> combined (128, T) tiles (xt-scoped) ----
                qk_c = [xp.tile([128, T], BF16, tag=f"qk_c{s}", name=f"qk_c{s}") for s in range(2)]
                v_c = xp.tile([128, T], BF16, tag="v_c", name="v_c")

                def qkv_mat(dst, lhsT_chunks, r1_trow, r1_s1, c1col):
                    for c4 in range(4):
                        sl = slice(c4 * 512, (c4 + 1) * 512)
                        ps = ppm.tile([128, 512], F32, tag="mm", name="mm")
                        for kc in range(3):
                            _mm(ps[:, :], lhsT_chunks[kc][:, :], xT[kc][:, sl],
                                start=(kc == 0), stop=False)
                        _mm(ps[:, :], r1_trow, bneg_row[0:1, sl], start=False, stop=False)
                        _mm(ps[:, :], r1_s1, mu_row[0:1, sl], start=False, stop=True)
                        tmp = xp.tile([128, 512], F32, tag=f"qtmp{c4 % 2}", name=f"qtmp{c4 % 2}")
                        nc.vector.tensor_tensor(tmp[:, :], ps[:, :], rstd_bc[:, sl], ALU.mult)
                        nc.scalar.activation(dst[:, sl], tmp[:, :], AF.Identity,
                                             bias=c1col, scale=1.0)

                for s in range(2):
                    qkv_mat(qk_c[s], wqk[s], r1qk[0:1, (2 * s) * 128:(2 * s) * 128 + 128],
                            r1qk[0:1, (2 * s + 1) * 128:(2 * s + 1) * 128 + 128], c1qk[:, s:s + 1])
                qkv_mat(v_c, wv, r1v[0:1, 0:128], r1v[0:1, 128:256], c1v[:, 0:1])

                # extract base-0 copies (all bf16 now, so plain DMAs off-engine)
                vA = xp.tile([64, T], BF16, tag="vA", name="vA")
                vB = xp.tile([64, T], BF16, tag="vB", name="vB")
                for s in range(2):
                    nc.gpsimd.dma_start(out=qT[s][:, :], in_=qk_c[s][0:64, :])
                    nc.gpsimd.dma_start(out=kT[s][:, :], in_=qk_c[s][64:128, :])
                nc.gpsimd.dma_start(out=vA[:, :], in_=v_c[0:64, :])
                nc.gpsimd.dma_start(out=vB[:, :], in_=v_c[64:128, :])
                # v row-major tiles: vrow[s][:, jt*64:(jt+1)*64] = v[jt-chunk].T
                for s, vsrc in ((0, vA), (1, vB)):
                    for g0 in range(0, NT, 4):
                        tr = ppb.tile([128, 512], BF16, tag="trb", name="trb")
                        for gi in range(4):
                            jt = g0 + gi
                            nc.tensor.transpose(tr[:, gi * 128:gi * 128 + 64],
                                                vsrc[:, jt * 128:(jt + 1) * 128], identb[0:64, 0:64])
                        for gi in range(4):
                            nc.scalar.copy(vrow[s][:, (g0 + gi) * 64:(g0 + gi + 1) * 64],
                                           tr[:, gi * 128:gi * 128 + 64])

            # ---------------- phase 3: attention per slot ----------------
            # 1-round sinkhorn (matches 6-iter reference to ~2e-6):
            #   alpha = 1/(rowsum exp(c)) = 1/(sacc + (T-L))   [T*a, from exp accum]
            #   b = 1/(S'^T alpha + sum(alpha))                [one matvec pass]
            #   y^T = alpha ∘ (S'(b∘V) + colsum(b∘V))
            with (
                tc.tile_pool(name="sp", bufs=1) as spp,
                tc.tile_pool(name="spt", bufs=1) as sptp,
                tc.tile_pool(name="att_misc", bufs=1) as amp,
            ):
                cbias = amp.tile([128, NT], F32, tag="cbias", name="cbias")
                nc.sync.dma_start(out=cbias[:, :], in_=cbias_d[:, :])
                for s in range(2):
                    sp = [spp.tile([128, (it + 1) * 128], BF16, tag=f"sp{s}{it}", name=f"sp{s}{it}") for it in range(NT)]
                    spt = [sptp.tile([128, (NT - jt) * 128], BF16, tag=f"spt{s}{jt}", name=f"spt{s}{jt}") for jt in range(NT)]
                    e = [spt[NT - 1 - it] for it in range(NT)]  # aliases (same size, bf16)

                    zall = amp.tile([128, NT], F32, tag=f"zall{s}", name=f"zall{s}")
                    for it in range(NT):
                        L = (it + 1) * 128
                        d0 = it * 128
                        nch = (L + 511) // 512
                        for c4 in range(nch):
                            lo, hi = c4 * 512, min(L, (c4 + 1) * 512)
                            ps = ppm.tile([128, 512], F32, tag="mm", name="mm")
                            _mm(ps[:, 0:hi - lo], qT[s][:, d0:d0 + 128], kT[s][:, lo:hi],
                                start=True, stop=True)
                            nc.scalar.activation(e[it][:, lo:hi], ps[:, 0:hi - lo],
                                                 AF.Exp, scale=0.125)
                        nc.gpsimd.affine_select(out=e[it][:, d0:L], in_=e[it][:, d0:L],
                                                compare_op=ALU.is_ge, fill=0.0, base=0,
                                                pattern=[[-1, 128]], channel_multiplier=1)
                        nc.vector.tensor_reduce(zall[:, it:it + 1], e[it][:, 0:L],
                                                axis=AXX, op=ALU.add)
                    rz = amp.tile([128, NT], F32, tag=f"rz{s}", name=f"rz{s}")
                    nc.vector.reciprocal(rz[:, :], zall[:, :])

                    # sp = exp(rz*E) - 1; accum gives rowsum(exp(c)) over stored cols
                    sacc = amp.tile([128, NT], F32, tag=f"sacc{s}", name=f"sacc{s}")
                    for it in range(NT):
                        L = (it + 1) * 128
                        nc.scalar.activation(sp[it][:, :], e[it][:, 0:L], AF.Exp,
                                             scale=rz[:, it:it + 1],
                                             accum_out=sacc[:, it:it + 1])
                        nc.vector.tensor_scalar(sp[it][:, :], sp[it][:, :], -1.0, None, ALU.add)

                    # alpha = 1/(sacc + (T - L)); bounce to row form (off critical path)
                    alpha = amp.tile([128, NT], F32R, tag=f"alpha{s}", name=f"alpha{s}")
                    nc.vector.tensor_tensor(alpha[:, :], sacc[:, :], cbias[:, :], ALU.add)
                    nc.vector.reciprocal(alpha[:, :], alpha[:, :])
                    al16 = amp.tile([128, NT], BF16, tag=f"al16{s}", name=f"al16{s}")
                    nc.vector.tensor_copy(al16[:, :], alpha[:, :])
                    nc.sync.dma_start(out=bnc_pview[s], in_=al16[:, :])
                    arow = amp.tile([1, T], BF16, tag="arow", name="arow")
                    nc.sync.dma_start(out=arow[0:1, :], in_=bounce[s:s + 1, :])

                    # transposes: sp (bf16) -> spt (bf16); copies 1/3 scalar, 2/3 vector
                    ncopy = 0
                    for jt in range(NT):
                        nit = NT - jt
                        for g0 in range(0, nit, 4):
                            gn = min(4, nit - g0)
                            tr = ppb.tile([128, 512], BF16, tag="trb", name="trb")
                            for gi in range(gn):
                                it = jt + g0 + gi
                                nc.tensor.transpose(tr[:, gi * 128:(gi + 1) * 128],
                                                    sp[it][:, jt * 128:(jt + 1) * 128],
                                                    identb[:, :])
                            if ncopy % 3 == 0:
                                nc.scalar.copy(spt[jt][:, g0 * 128:(g0 + gn) * 128], tr[:, 0:gn * 128])
                            else:
                                nc.vector.tensor_copy(spt[jt][:, g0 * 128:(g0 + gn) * 128], tr[:, 0:gn * 128])
                            ncopy += 1

                    # ---- one matvec pass: r = S'^T alpha (row form), bank-outer ----
                    ared = amp.tile([128, 1], F32, tag=f"ared{s}", name=f"ared{s}")
                    nc.vector.tensor_reduce(ared[:, :], alpha[:, :], axis=AXX, op=ALU.add)
                    ps1 = ppm.tile([1, 512], F32, tag="mm", name="mm")
                    _mm(ps1[0:1, 0:1], onescf[:, :], ared[:, :], start=True, stop=True)
                    asum = amp.tile([1, 1], F32, tag=f"asum{s}", name=f"asum{s}")
                    nc.scalar.copy(asum[0:1, :], ps1[0:1, 0:1])
                    brow = amp.tile([1, T], F32, tag="brow", name="brow")
                    bps = ppt.tile([128, 512], F32, tag="tr", name="tr")
                    for c4 in range(4):
                        lo, hi = c4 * 512, (c4 + 1) * 512
                        wps = ppw.tile([64, 512], F32, tag="wide", name="wide")
                        for it in range(4 * c4, NT):
                            L = (it + 1) * 128
                            shi = min(L, hi) - lo
                            _mm(wps[0:1, 0:shi], al16[:, it:it + 1], sp[it][:, lo:lo + shi],
                                start=(it == c4 * 4), stop=(it == NT - 1))
                        # brow holds r + sum(alpha); reciprocal happens in column space
                        nc.scalar.activation(brow[0:1, lo:hi], wps[0:1, :], AF.Identity,
                                             bias=asum[0:1, 0:1], scale=1.0)
                        for jq in range(4):
                            jt = 4 * c4 + jq
                            nc.tensor.transpose(bps[:, jt:jt + 1],
                                                brow[0:1, jt * 128:(jt + 1) * 128],
                                                ident[0:1, 0:1])
                    bcol = amp.tile([128, NT], F32, tag=f"bcol{s}", name=f"bcol{s}")
                    nc.scalar.copy(bcol[:, :], bps[:, 0:NT])
                    nc.vector.reciprocal(bcol[:, :], bcol[:, :])
                    bv = []
                    for jt in range(NT):
                        bvt = amp.tile([128, 64], BF16, tag=f"bv{jt}", name=f"bv{jt}")
                        nc.vector.tensor_scalar(bvt[:, :], vrow[s][:, jt * 64:(jt + 1) * 64],
                                                bcol[:, jt:jt + 1], None, ALU.mult)
                        bv.append(bvt)

                    # ---- y^T = alpha ∘ (S' @ (b∘V) + colsum(b∘V)), bank-outer ----
                    wcps = ppt.tile([128, 512], F32, tag="tr", name="tr")
                    for jt in range(NT):
                        _mm(wcps[0:1, 0:64], onescb[:, :], bv[jt][:, :],
                            start=(jt == 0), stop=(jt == NT - 1))
                    wrow = amp.tile([1, 64], F32, tag=f"wrow{s}", name=f"wrow{s}")
                    nc.scalar.copy(wrow[0:1, :], wcps[0:1, 0:64])
                    wtp = ppt.tile([128, 512], F32, tag="tr", name="tr")
                    nc.tensor.transpose(wtp[0:64, 0:1], wrow[0:1, :], ident[0:1, 0:1])
                    tw = amp.tile([64, 1], F32, tag=f"tw{s}", name=f"tw{s}")
                    nc.scalar.copy(tw[:, :], wtp[0:64, 0:1])
                    for c4 in range(4):
                        lo, hi = c4 * 512, (c4 + 1) * 512
                        sl = slice(lo, hi)
                        yps = ppw.tile([64, 512], F32, tag="wide", name="wide")
                        for jt in range(0, min(NT, 4 * c4 + 4)):
                            j0 = jt * 128
                            slo = max(lo, j0)
                            _mmb(yps[:, slo - lo:512], bv[jt][:, :],
                                 spt[jt][:, slo - j0:hi - j0],
                                 start=(jt == 0), stop=(jt == min(NT - 1, 4 * c4 + 3)))
                        psa = ppm.tile([128, 512], F32, tag="mm", name="mm")
                        _mm(psa[0:64, :], onesrb[0:1, 0:64], arow[0:1, sl], start=True, stop=True)
                        abc = amp.tile([64, 512], F32R, tag="abc", name="abc")
                        nc.scalar.copy(abc[:, :], psa[0:64, :])
                        ytmp = amp.tile([64, 512], BF16, tag="ytmp", name="ytmp")
                        nc.scalar.activation(ytmp[:, :], yps[:, :], AF.Identity,
                                             bias=tw[:, 0:1], scale=1.0)
                        nc.vector.tensor_tensor(ytmp[:, :], ytmp[:, :], abc[:, :], ALU.mult)
                        for grp in range(2):
                            nc.gpsimd.dma_start(out=a2a_in[s][grp * 4 + c4, :, :],
                                                in_=ytmp[:, :])

                    # per-slot AllToAll: slot 0's overlaps slot 1's compute
                    nc.gpsimd.collective_compute(
                        "AllToAll", ALU.bypass,
                        replica_groups=[list(range(N_CORES))],
                        ins=[a2a_in[s].opt()],
                        outs=[a2a_out[s].opt()],
                    )

            # ---------------- phase 5: proj + LN2 + MLP ----------------
            with tc.tile_pool(name="tail", bufs=1) as tp:
                stk0 = [tp.tile([128, 512], BF16, tag=f"stk0{h}", name=f"stk0{h}") for h in range(H)]
                for h in range(H):
                    c0, s0 = UNIT_SLOT[h]
                    c1_, s1_ = UNIT_SLOT[H + h]
                    nc.gpsimd.dma_start(out=stk0[h][0:64, :], in_=a2a_out[s0][c0, :, :])
                    nc.gpsimd.dma_start(out=stk0[h][64:128, :], in_=a2a_out[s1_][c1_, :, :])

                hT = [tp.tile([128, 512], F32R, tag=f"ht{ec}", name=f"ht{ec}") for ec in range(3)]
                for ec in range(3):
                    ps = ppm.tile([128, 512], F32, tag="mm", name="mm")
                    for h in range(H):
                        _mm(ps[:, :], wproj[h][ec], stk0[h][:, :],
                            start=(h == 0), stop=(h == H - 1))
                    nc.scalar.activation(hT[ec][:, :], ps[:, :], AF.Identity,
                                         bias=bproj[:, ec:ec + 1], scale=1.0)

                mu2ps = ppm.tile([1, 512], F32, tag="mm", name="mm")
                for ec in range(3):
                    _mm(mu2ps[0:1, :], onesc[:, :], hT[ec][:, :], start=(ec == 0), stop=(ec == 2))
                mu2r = tp.tile([1, 512], F32R, tag="mu2r", name="mu2r")
                nc.scalar.activation(mu2r[0:1, :], mu2ps[0:1, :], AF.Identity,
                                     bias=sbias[0:1, 0:1], scale=1.0 / CP1)
                scr2 = tp.tile([128, 512], F32R, tag="scr2", name="scr2")
                msq2ps = ppm.tile([1, 512], F32, tag="mm", name="mm")
                for ec in range(3):
                    nc.scalar.square(scr2[:, :], hT[ec][:, :])
                    _mm(msq2ps[0:1, :], onesc[:, :], scr2[:, :], start=(ec == 0), stop=(ec == 2))
                msq2r = tp.tile([1, 512], F32, tag="msq2r", name="msq2r")
                nc.scalar.activation(msq2r[0:1, :], msq2ps[0:1, :], AF.Identity,
                                     bias=sbias[0:1, 1:2], scale=1.0 / CP1)
                v2r = tp.tile([1, 512], F32R, tag="v2r", name="v2r")
                nc.vector.tensor_tensor(v2r[0:1, :], mu2r[0:1, :], mu2r[0:1, :], ALU.mult)
                nc.vector.tensor_tensor(v2r[0:1, :], msq2r[0:1, :], v2r[0:1, :], ALU.subtract)
                nc.scalar.activation(v2r[0:1, :], v2r[0:1, :], AF.Ln, bias=epsc[0:1, 0:1])
                # rstd2 = Exp(-0.5*ln(var+eps)) straight from the broadcast psum
                ps = ppm.tile([128, 512], F32, tag="mm", name="mm")
                _mm(ps[:, :], onesr[:, :], v2r[0:1, :], start=True, stop=True)
                rstd2bc = tp.tile([128, 512], F32, tag="rstd2bc", name="rstd2bc")
                nc.scalar.activation(rstd2bc[:, :], ps[:, :], AF.Exp, scale=-0.5)
                rstd2r = tp.tile([1, 512], F32R, tag="rstd2r", name="rstd2r")
                nc.vector.tensor_copy(rstd2r[0:1, :], rstd2bc[0:1, :])
                m2rr = tp.tile([1, 512], BF16, tag="m2rr", name="m2rr")
                b2rr = tp.tile([1, 512], BF16, tag="b2rr", name="b2rr")
                nc.vector.tensor_tensor(m2rr[0:1, :], mu2r[0:1, :], rstd2r[0:1, :], ALU.mult)
                b2f = tp.tile([1, 512], F32R, tag="b2f", name="b2f")
                nc.vector.tensor_scalar(b2f[0:1, :], mu2r[0:1, :], tcol[0:1, 0:1], None, ALU.subtract)
                nc.vector.tensor_tensor(b2rr[0:1, :], b2f[0:1, :], rstd2r[0:1, :], ALU.mult)
                hs = [tp.tile([128, 512], BF16, tag=f"hs{ec}", name=f"hs{ec}") for ec in range(3)]
                for ec in range(3):
                    nc.vector.tensor_tensor(hs[ec][:, :], hT[ec][:, :], rstd2bc[:, :], ALU.mult)

                mT = [tp.tile([128, 512], BF16, tag=f"mt{jc}", name=f"mt{jc}") for jc in range(12)]
                for jc in range(12):
                    ps = ppm.tile([128, 512], F32, tag="mm", name="mm")
                    for kc in range(3):
                        _mm(ps[:, :], wf[jc][kc], hs[kc][:, :], start=(kc == 0), stop=False)
                    _mm(ps[:, :], ns2f[0:1, jc * 128:(jc + 1) * 128], m2rr[0:1, :], start=False, stop=False)
                    _mm(ps[:, :], nwft[0:1, jc * 128:(jc + 1) * 128], b2rr[0:1, :], start=False, stop=True)
                    nc.scalar.activation(mT[jc][:, :], ps[:, :], AF.Gelu,
                                         bias=c2b[:, jc:jc + 1], scale=1.0)
                for ec in range(3):
                    ps = ppm.tile([128, 512], F32, tag="mm", name="mm")
                    for kc in range(12):
                        _mm(ps[:, :], wf2[ec][kc], mT[kc][:, :],
                            start=(kc == 0), stop=(kc == 11))
                    oT = tp.tile([128, 512], F32, tag=f"ot{ec}", name=f"ot{ec}")
                    nc.scalar.activation(oT[:, :], ps[:, :], AF.Identity,
                                         bias=bfc2[:, ec:ec + 1], scale=1.0)
                    nc.sync.dma_start(out=out_d[ec * 128:(ec + 1) * 128, :], in_=oT[:, :])

    nc.compile()
    return nc


def host_prep(inputs):
    x = np.asarray(inputs["x"], np.float32)
    t = float(np.asarray(inputs["t"]).reshape(-1)[0])
    w1 = np.asarray(inputs["ln1_w"], np.float32); b1 = np.asarray(inputs["ln1_b"], np.float32)
    Wa = np.asarray(inputs["attn_w"], np.float32); ba = np.asarray(inputs["attn_b"], np.float32)
    Wp_ = w1[:, None] * Wa
    c1 = b1 @ Wa + ba
    Wa_main, Wa_trow = Wp_[:C], Wp_[C]
    s1 = Wp_[:C].sum(axis=0)
    w2 = np.asarray(inputs["ln2_w"], np.float32); b2 = np.asarray(inputs["ln2_b"], np.float32)
    Wf = np.asarray(inputs["fc_w"], np.float32); bf = np.asarray(inputs["fc_b"], np.float32)
    Wf_p = w2[:, None] * Wf
    c2 = b2 @ Wf + bf
    Wf_main, Wf_trow = Wf_p[:C], Wf_p[C]
    s2f = Wf_p[:C].sum(axis=0)
    Wpj = np.asarray(inputs["proj_w"], np.float32); bpj = np.asarray(inputs["proj_b"], np.float32)
    Wf2 = np.asarray(inputs["fc2_w"], np.float32); bf2 = np.asarray(inputs["fc2_b"], np.float32)

    common = {
        "ident": np.eye(128, dtype=np.float32),
        "onesc": np.ones((128, 1), np.float32),
        "onesr": np.ones((1, 128), np.float32),
        "tcol": np.full((128, 1), t, np.float32),
        "sbias": np.array([[t / CP1, t * t / CP1]], np.float32),
        "epsc": np.full((128, 1), EPS, np.float32),
        "cbias": np.broadcast_to(
            np.array([T - (it + 1) * 128 for it in range(NT)], np.float32),
            (128, NT)).copy(),
        "bproj": bpj.reshape(3, 128).T.astype(np.float32).copy(),
        "c2b": c2.reshape(12, 128).T.astype(np.float32).copy(),
        "bfc2": bf2.reshape(3, 128).T.astype(np.float32).copy(),
        "nwft": (-Wf_trow)[None, :].astype(BF16NP).copy(),
        "ns2f": (-s2f)[None, :].astype(BF16NP).copy(),
        "wf": np.stack([np.stack([Wf_main[kc * 128:(kc + 1) * 128, jc * 128:(jc + 1) * 128]
                                  for kc in range(3)]) for jc in range(12)])
              .transpose(2, 0, 1, 3).reshape(128, -1).astype(BF16NP).copy(),
        "wf2": np.stack([np.stack([Wf2[kc * 128:(kc + 1) * 128, ec * 128:(ec + 1) * 128]
                                   for kc in range(12)]) for ec in range(3)])
               .transpose(2, 0, 1, 3).reshape(128, -1).astype(BF16NP).copy(),
    }

    in_maps = []
    for c in range(N_CORES):
        units = CORE_UNITS[c]
        myb = UNITS[units[0]][0]
        m = dict(common)
        m["xT"] = np.ascontiguousarray(x[myb].T)
        shard_b = c // 4  # batch of the row shard this core finishes (receiver side)
        wproj = np.zeros((H, 3, 128, 128), np.float32)
        for h in range(H):
            for ec in range(3):
                blk = Wpj[h * HD:(h + 1) * HD, ec * 128:(ec + 1) * 128]
                if shard_b == 0:
                    wproj[h, ec, 0:64] = blk
                else:
                    wproj[h, ec, 64:128] = blk
        m["wproj"] = wproj.transpose(2, 0, 1, 3).reshape(128, -1).astype(BF16NP).copy()
        wqk = np.zeros((2, 3, 128, 128), np.float32)
        r1qk = np.zeros((1, 512), np.float32)
        c1qk = np.zeros((128, 2), np.float32)
        wv = np.zeros((3, 128, 128), np.float32)
        r1v = np.zeros((1, 256), np.float32)
        c1v = np.zeros((128, 1), np.float32)
        for s, u in enumerate(units):
            _, h = UNITS[u]
            cq = slice(h * HD, (h + 1) * HD)
            ck = slice(C + h * HD, C + (h + 1) * HD)
            cv = slice(2 * C + h * HD, 2 * C + (h + 1) * HD)
            for kc in range(3):
                wqk[s, kc, :, 0:64] = Wa_main[kc * 128:(kc + 1) * 128, cq]
                wqk[s, kc, :, 64:128] = Wa_main[kc * 128:(kc + 1) * 128, ck]
                wv[kc, :, s * 64:(s + 1) * 64] = Wa_main[kc * 128:(kc + 1) * 128, cv]
            base = 2 * s * 128
            r1qk[0, base:base + 64] = -Wa_trow[cq]; r1qk[0, base + 64:base + 128] = -Wa_trow[ck]
            r1qk[0, base + 128:base + 192] = -s1[cq]; r1qk[0, base + 192:base + 256] = -s1[ck]
            r1v[0, s * 64:(s + 1) * 64] = -Wa_trow[cv]
            r1v[0, 128 + s * 64:128 + (s + 1) * 64] = -s1[cv]
            c1qk[0:64, s] = c1[cq]; c1qk[64:128, s] = c1[ck]
            c1v[s * 64:(s + 1) * 64, 0] = c1[cv]
        m["wqk"] = wqk; m["r1qk"] = r1qk; m["c1qk"] = c1qk
        m["wv"] = wv; m["r1v"] = r1v; m["c1v"] = c1v
        in_maps.append(m)
    return in_maps


def kernel(**inputs):
    if "nc" not in _COMPILED:
        _COMPILED["nc"] = build_program()
    nc = _COMPILED["nc"]
    in_maps = host_prep(inputs)
    res = run_bass_kernel_spmd(nc, in_maps, list(range(N_CORES)))
    out = np.zeros((B, T, C), np.float32)
    for c in range(N_CORES):
        oT = res.results[c]["oT"]
        b, t0 = c // 4, (c % 4) * 512
        out[b, t0:t0 + 512, :] = oT.T
    return out



# revision 83
# speedup vs baseline: 1.1139x; 1.0066x over previous
"""Trainium2 Bass kernel for nn_Block_87428354277599 (sinkhorn-attention transformer block).

Self-contained: hardcodes shapes/sharding. kernel(**inputs) -> (2, 2048, 384) f32.

Sharding (8 cores, SPMD):
- 12 (batch, head) units padded to 16 slots: every core runs 2 attention slots
  (cores 4-7's slot 1 gets zero weights; its junk output is never consumed).
- LN1/LN2 are folded into the QKV / MLP matmuls via host-precomputed weight folds
  plus rank-1 corrections (mu and t-column terms) accumulated on the PE.
- Sinkhorn on the row-softmaxed causal attention == multiplicative matrix scaling
  of S = exp(P). S-1 is lower-triangular, so only the lower triangle (S' = S-1)
  is stored SBUF-resident in both layouts (S' f32, S'^T bf16); the all-ones part
  of S becomes global-sum corrections (kept f32). All matvecs run on the PE.
- y^T slices are exchanged with one AllToAll (each sender duplicates its slices
  into both batch shard groups; receivers mask the wrong batch via zeroed halves
  of the duplicated proj weights). proj+LN2+MLP run row-sharded (512 rows/core).
"""

import numpy as np
import ml_dtypes

BF16NP = ml_dtypes.bfloat16

import concourse.bacc as bacc
import concourse.mybir as mybir
from concourse.tile import TileContext
from concourse.bass_utils import run_bass_kernel_spmd

F32 = mybir.dt.float32
BF16 = mybir.dt.bfloat16
F32R = mybir.dt.float32r
AF = mybir.ActivationFunctionType
ALU = mybir.AluOpType
AXX = mybir.AxisListType.X

B, T, C, H, HD = 2, 2048, 384, 6, 64
CP1 = C + 1
N_CORES = 8
NT = T // 128  # 16
EPS = 1e-5
UNITS = [(u // H, u % H) for u in range(2 * H)]  # 12 real units
CORE_UNITS = {0: [0, 1], 1: [2, 3], 2: [4, 5], 3: [6, 7], 4: [8], 5: [9], 6: [10], 7: [11]}
UNIT_SLOT = {}
for _c, _us in CORE_UNITS.items():
    for _s, _u in enumerate(_us):
        UNIT_SLOT[_u] = (_c, _s)

_COMPILED = {}


def build_program():
    nc = bacc.Bacc(trn_type="TRN2", num_devices=N_CORES)

    def _mm(out, lhsT, rhs, start, stop):
        nc.tensor.matmul(out, lhsT, rhs, start=start, stop=stop)

    _mmb = _mm

    def din(name, shape, dt=F32):
        return nc.dram_tensor(name, list(shape), dt, kind="ExternalInput")

    xT_d = din("xT", (C, T), F32R)
    wqk_d = din("wqk", (2, 3, 128, 128), F32R)
    wv_d = din("wv", (3, 128, 128), F32R)
    r1qk_d = din("r1qk", (1, 512), F32R)
    r1v_d = din("r1v", (1, 256), F32R)
    c1qk_d = din("c1qk", (128, 2))
    c1v_d = din("c1v", (128, 1))
    ident_d = din("ident", (128, 128))
    onesc_d = din("onesc", (128, 1), F32R)
    onesr_d = din("onesr", (1, 128), F32R)
    tcol_d = din("tcol", (128, 1))
    sbias_d = din("sbias", (1, 2))
    epsc_d = din("epsc", (128, 1))
    cbias_d = din("cbias", (128, NT))
    wproj_d = din("wproj", (128, H * 3 * 128), BF16)
    bproj_d = din("bproj", (128, 3))
    wf_d = din("wf", (128, 12 * 3 * 128), BF16)
    nwft_d = din("nwft", (1, 1536), BF16)
    ns2f_d = din("ns2f", (1, 1536), BF16)
    c2b_d = din("c2b", (128, 12))
    wf2_d = din("wf2", (128, 3 * 12 * 128), BF16)
    bfc2_d = din("bfc2", (128, 3))
    out_d = nc.dram_tensor("oT", [C, 512], F32, kind="ExternalOutput")

    with TileContext(nc) as tc, nc.allow_low_precision(reason="f32r-typed intermediates (same bits as f32)"):
        with (
            tc.tile_pool(name="const", bufs=1) as cpool,
            tc.tile_pool(name="dram", bufs=1, space="DRAM") as dpool,
            tc.tile_pool(name="ps_wide", bufs=2, space="PSUM") as ppw,
            tc.tile_pool(name="ps_mm", bufs=3, space="PSUM") as ppm,
            tc.tile_pool(name="ps_tr", bufs=1, space="PSUM") as ppt,
            tc.tile_pool(name="ps_trb", bufs=2, space="PSUM") as ppb,
            tc.tile_pool(name="vec", bufs=1) as vp,
            tc.tile_pool(name="qk", bufs=1) as qkp,
            tc.tile_pool(name="tailw", bufs=1) as twp,
        ):
            a2a_in = [dpool.tile([8, 64, 512], BF16, name=f"a2a_in{s}") for s in range(2)]
            a2a_out = [dpool.tile([8, 64, 512], BF16, name=f"a2a_out{s}") for s in range(2)]
            bounce = dpool.tile([2, T], BF16, name="bounce")
            bnc_pview = [bounce[s:s + 1, :].rearrange("a (f p) -> (a p) f", p=128)
                         for s in range(2)]

            ident = cpool.tile([128, 128], F32, tag="ident", name="ident")
            onesc = cpool.tile([128, 1], F32R, tag="onesc", name="onesc")
            onesr = cpool.tile([1, 128], F32R, tag="onesr", name="onesr")
            tcol = cpool.tile([128, 1], F32, tag="tcol", name="tcol")
            sbias = cpool.tile([1, 2], F32, tag="sbias", name="sbias")
            epsc = cpool.tile([128, 1], F32, tag="epsc", name="epsc")
            nc.sync.dma_start(out=ident[:, :], in_=ident_d[:, :])
            nc.sync.dma_start(out=onesc[:, :], in_=onesc_d[:, :])
            nc.sync.dma_start(out=onesr[:, :], in_=onesr_d[:, :])
            nc.sync.dma_start(out=tcol[:, :], in_=tcol_d[:, :])
            nc.sync.dma_start(out=sbias[:, :], in_=sbias_d[:, :])
            nc.sync.dma_start(out=epsc[:, :], in_=epsc_d[:, :])
            identr = cpool.tile([128, 128], F32R, tag="identr", name="identr")
            nc.scalar.copy(identr[:, :], ident[:, :])
            onescf = cpool.tile([128, 1], F32, tag="onescf", name="onescf")
            onesrf = cpool.tile([1, 128], F32, tag="onesrf", name="onesrf")
            nc.scalar.copy(onescf[:, :], onesc[:, :])
            nc.scalar.copy(onesrf[:, :], onesr[:, :])
            identb = cpool.tile([128, 128], BF16, tag="identb", name="identb")
            nc.scalar.copy(identb[:, :], ident[:, :])
            onescb = cpool.tile([128, 1], BF16, tag="onescb", name="onescb")
            nc.scalar.copy(onescb[:, :], onesc[:, :])
            onesrb = cpool.tile([1, 128], BF16, tag="onesrb", name="onesrb")
            nc.scalar.copy(onesrb[:, :], onesr[:, :])

            # persistent per-slot activations (base-partition-0 tiles)
            qT = [qkp.tile([64, T], BF16, tag=f"qT{s}", name=f"qT{s}") for s in range(2)]
            kT = [qkp.tile([64, T], BF16, tag=f"kT{s}", name=f"kT{s}") for s in range(2)]
            vrow = [qkp.tile([128, NT * 64], BF16, tag=f"vrow{s}", name=f"vrow{s}") for s in range(2)]

            # ---------------- phase 1+2: stats + QKV (xt-scoped) ----------------
            with tc.tile_pool(name="xt", bufs=1) as xp:
                xT = [xp.tile([128, T], F32R, tag=f"xt{kc}", name=f"xt{kc}") for kc in range(3)]
                for kc in range(3):
                    nc.sync.dma_start(out=xT[kc][:, :], in_=xT_d[kc * 128:(kc + 1) * 128, :])
                wqk = [[xp.tile([128, 128], F32R, tag=f"wqk{s}{kc}", name=f"wqk{s}{kc}") for kc in range(3)] for s in range(2)]
                wv = [xp.tile([128, 128], F32R, tag=f"wv{kc}", name=f"wv{kc}") for kc in range(3)]
                r1qk = xp.tile([1, 512], F32R, tag="r1qk", name="r1qk")
                r1v = xp.tile([1, 256], F32R, tag="r1v", name="r1v")
                c1qk = xp.tile([128, 2], F32, tag="c1qk", name="c1qk")
                c1v = xp.tile([128, 1], F32, tag="c1v", name="c1v")
                for s in range(2):
                    for kc in range(3):
                        nc.sync.dma_start(out=wqk[s][kc][:, :], in_=wqk_d[s, kc, :, :])
                for kc in range(3):
                    nc.sync.dma_start(out=wv[kc][:, :], in_=wv_d[kc, :, :])
                nc.sync.dma_start(out=r1qk[:, :], in_=r1qk_d[:, :])
                nc.sync.dma_start(out=r1v[:, :], in_=r1v_d[:, :])
                nc.sync.dma_start(out=c1qk[:, :], in_=c1qk_d[:, :])
                nc.sync.dma_start(out=c1v[:, :], in_=c1v_d[:, :])

                # tail weights (bf16): single mega-tile DMAs, hide under attention
                wproj_a = twp.tile([128, H * 3 * 128], BF16, tag="wproj_a", name="wproj_a")
                wf_a = twp.tile([128, 12 * 3 * 128], BF16, tag="wf_a", name="wf_a")
                wf2_a = twp.tile([128, 3 * 12 * 128], BF16, tag="wf2_a", name="wf2_a")
                bproj = twp.tile([128, 3], F32, tag="bproj", name="bproj")
                nwft = twp.tile([1, 1536], BF16, tag="nwft", name="nwft")
                ns2f = twp.tile([1, 1536], BF16, tag="ns2f", name="ns2f")
                c2b = twp.tile([128, 12], F32, tag="c2b", name="c2b")
                bfc2 = twp.tile([128, 3], F32, tag="bfc2", name="bfc2")
                nc.sync.dma_start(out=wproj_a[:, :], in_=wproj_d[:, :])
                nc.sync.dma_start(out=wf_a[:, :], in_=wf_d[:, :])
                nc.sync.dma_start(out=wf2_a[:, :], in_=wf2_d[:, :])
                wproj = [[wproj_a[:, (h * 3 + ec) * 128:(h * 3 + ec + 1) * 128]
                          for ec in range(3)] for h in range(H)]
                wf = [[wf_a[:, (jc * 3 + kc) * 128:(jc * 3 + kc + 1) * 128]
                       for kc in range(3)] for jc in range(12)]
                wf2 = [[wf2_a[:, (ec * 12 + kc) * 128:(ec * 12 + kc + 1) * 128]
                        for kc in range(12)] for ec in range(3)]
                nc.sync.dma_start(out=bproj[:, :], in_=bproj_d[:, :])
                nc.sync.dma_start(out=nwft[:, :], in_=nwft_d[:, :])
                nc.sync.dma_start(out=ns2f[:, :], in_=ns2f_d[:, :])
                nc.sync.dma_start(out=c2b[:, :], in_=c2b_d[:, :])
                nc.sync.dma_start(out=bfc2[:, :], in_=bfc2_d[:, :])

                # ---- stats, fully chunk-local so qkv can chase per c4 ----
                mu_row = xp.tile([1, T], F32R, tag="mu_row", name="mu_row")
                msq_row = xp.tile([1, T], F32, tag="msq_row", name="msq_row")
                var_row = xp.tile([1, T], F32R, tag="var_row", name="var_row")
                bneg_row = xp.tile([1, T], F32R, tag="bneg_row", name="bneg_row")
                rstd_bc = xp.tile([128, T], F32, tag="rstd_bc", name="rstd_bc")
                for c4 in range(4):
                    sl = slice(c4 * 512, (c4 + 1) * 512)
                    wide = ppw.tile([64, 512], F32, tag="wide", name="wide")
                    for kc in range(3):
                        _mm(wide[0:1, :], onesc[:, :],
                            xT[kc][:, sl], start=(kc == 0), stop=(kc == 2))
                    nc.scalar.activation(mu_row[0:1, sl], wide[0:1, :],
                                         AF.Identity, bias=sbias[0:1, 0:1], scale=1.0 / CP1)
                    ps = ppm.tile([1, 512], F32, tag="mm", name="mm")
                    for kc in range(3):
                        sq = xp.tile([128, 512], F32R, tag=f"scr{c4 % 2}", name=f"scr{c4 % 2}")
                        nc.scalar.square(sq[:, :], xT[kc][:, sl])
                        _mm(ps[0:1, :], onesc[:, :], sq[:, :], start=(kc == 0), stop=(kc == 2))
                    nc.scalar.activation(msq_row[0:1, sl], ps[0:1, :],
                                         AF.Identity, bias=sbias[0:1, 1:2], scale=1.0 / CP1)
                    nc.vector.tensor_tensor(var_row[0:1, sl], mu_row[0:1, sl],
                                            mu_row[0:1, sl], ALU.mult)
                    nc.vector.tensor_tensor(var_row[0:1, sl], msq_row[0:1, sl],
                                            var_row[0:1, sl], ALU.subtract)
                    # lnv = Ln(var+eps); rstd = Exp(-0.5*lnv) — all scalar, no DVE recip
                    nc.scalar.activation(var_row[0:1, sl], var_row[0:1, sl],
                                         AF.Ln, bias=epsc[0:1, 0:1])
                    nc.vector.tensor_scalar(bneg_row[0:1, sl], mu_row[0:1, sl],
                                            tcol[0:1, 0:1], None, ALU.subtract)
                    psb = ppm.tile([128, 512], F32, tag="mm", name="mm")
                    _mm(psb[:, :], onesr[:, :], var_row[0:1, sl], start=True, stop=True)
                    nc.scalar.activation(rstd_bc[:, sl], psb[:, :], AF.Exp, scale=-0.5)

                # ---- QKV matmuls -> combined (128, T) tiles (xt-scoped) ----
                qk_c = [xp.tile([128, T], BF16, tag=f"qk_c{s}", name=f"qk_c{s}") for s in range(2)]
                v_c = xp.tile([128, T], BF16, tag="v_c", name="v_c")

                def qkv_mat(dst, lhsT_chunks, r1_trow, r1_s1, c1col):
                    for c4 in range(4):
                        sl = slice(c4 * 512, (c4 + 1) * 512)
                        ps = ppm.tile([128, 512], F32, tag="mm", name="mm")
                        for kc in range(3):
                            _mm(ps[:, :], lhsT_chunks[kc][:, :], xT[kc][:, sl],
                                start=(kc == 0), stop=False)
                        _mm(ps[:, :], r1_trow, bneg_row[0:1, sl], start=False, stop=False)
                        _mm(ps[:, :], r1_s1, mu_row[0:1, sl], start=False, stop=True)
                        tmp = xp.tile([128, 512], F32, tag=f"qtmp{c4 % 2}", name=f"qtmp{c4 % 2}")
                        nc.vector.tensor_tensor(tmp[:, :], ps[:, :], rstd_bc[:, sl], ALU.mult)
                        nc.scalar.activation(dst[:, sl], tmp[:, :], AF.Identity,
                                             bias=c1col, scale=1.0)

                for s in range(2):
                    qkv_mat(qk_c[s], wqk[s], r1qk[0:1, (2 * s) * 128:(2 * s) * 128 + 128],
                            r1qk[0:1, (2 * s + 1) * 128:(2 * s + 1) * 128 + 128], c1qk[:, s:s + 1])
                qkv_mat(v_c, wv, r1v[0:1, 0:128], r1v[0:1, 128:256], c1v[:, 0:1])

                # extract base-0 copies (all bf16 now, so plain DMAs off-engine)
                vA = xp.tile([64, T], BF16, tag="vA", name="vA")
                vB = xp.tile([64, T], BF16, tag="vB", name="vB")
                for s in range(2):
                    nc.gpsimd.dma_start(out=qT[s][:, :], in_=qk_c[s][0:64, :])
                    nc.gpsimd.dma_start(out=kT[s][:, :], in_=qk_c[s][64:128, :])
                nc.gpsimd.dma_start(out=vA[:, :], in_=v_c[0:64, :])
                nc.gpsimd.dma_start(out=vB[:, :], in_=v_c[64:128, :])
                # v row-major tiles: vrow[s][:, jt*64:(jt+1)*64] = v[jt-chunk].T
                for s, vsrc in ((0, vA), (1, vB)):
                    for g0 in range(0, NT, 4):
                        tr = ppb.tile([128, 512], BF16, tag="trb", name="trb")
                        for gi in range(4):
                            jt = g0 + gi
                            nc.tensor.transpose(tr[:, gi * 128:gi * 128 + 64],
                                                vsrc[:, jt * 128:(jt + 1) * 128], identb[0:64, 0:64])
                        for gi in range(4):
                            nc.scalar.copy(vrow[s][:, (g0 + gi) * 64:(g0 + gi + 1) * 64],
                                           tr[:, gi * 128:gi * 128 + 64])

            # ---------------- phase 3: attention per slot ----------------
            # 1-round sinkhorn (matches 6-iter reference to ~2e-6):
            #   alpha = 1/(rowsum exp(c)) = 1/(sacc + (T-L))   [T*a, from exp accum]
            #   b = 1/(S'^T alpha + sum(alpha))                [one matvec pass]
            #   y^T = alpha ∘ (S'(b∘V) + colsum(b∘V))
            with (
                tc.tile_pool(name="sp", bufs=1) as spp,
                tc.tile_pool(name="spt", bufs=1) as sptp,
                tc.tile_pool(name="att_misc", bufs=1) as amp,
            ):
                cbias = amp.tile([128, NT], F32, tag="cbias", name="cbias")
                nc.sync.dma_start(out=cbias[:, :], in_=cbias_d[:, :])
                for s in range(2):
                    sp = [spp.tile([128, (it + 1) * 128], BF16, tag=f"sp{s}{it}", name=f"sp{s}{it}") for it in range(NT)]
                    spt = [sptp.tile([128, (NT - jt) * 128], BF16, tag=f"spt{s}{jt}", name=f"spt{s}{jt}") for jt in range(NT)]
                    e = [spt[NT - 1 - it] for it in range(NT)]  # aliases (same size, bf16)

                    zall = amp.tile([128, NT], F32, tag=f"zall{s}", name=f"zall{s}")
                    for it in range(NT):
                        L = (it + 1) * 128
                        d0 = it * 128
                        nch = (L + 511) // 512
                        for c4 in range(nch):
                            lo, hi = c4 * 512, min(L, (c4 + 1) * 512)
                            ps = ppm.tile([128, 512], F32, tag="mm", name="mm")
                            _mm(ps[:, 0:hi - lo], qT[s][:, d0:d0 + 128], kT[s][:, lo:hi],
                                start=True, stop=True)
                            nc.scalar.activation(e[it][:, lo:hi], ps[:, 0:hi - lo],
                                                 AF.Exp, scale=0.125)
                        nc.gpsimd.affine_select(out=e[it][:, d0:L], in_=e[it][:, d0:L],
                                                compare_op=ALU.is_ge, fill=0.0, base=0,
                                                pattern=[[-1, 128]], channel_multiplier=1)
                        nc.vector.tensor_reduce(zall[:, it:it + 1], e[it][:, 0:L],
                                                axis=AXX, op=ALU.add)
                    rz = amp.tile([128, NT], F32, tag=f"rz{s}", name=f"rz{s}")
                    nc.vector.reciprocal(rz[:, :], zall[:, :])

                    # sp = exp(rz*E) - 1; accum gives rowsum(exp(c)) over stored cols
                    sacc = amp.tile([128, NT], F32, tag=f"sacc{s}", name=f"sacc{s}")
                    for it in range(NT):
                        L = (it + 1) * 128
                        nc.scalar.activation(sp[it][:, :], e[it][:, 0:L], AF.Exp,
                                             scale=rz[:, it:it + 1],
                                             accum_out=sacc[:, it:it + 1])
                        nc.vector.tensor_scalar(sp[it][:, :], sp[it][:, :], -1.0, None, ALU.add)

                    # alpha = 1/(sacc + (T - L)); bounce to row form (off critical path)
                    alpha = amp.tile([128, NT], F32R, tag=f"alpha{s}", name=f"alpha{s}")
                    nc.vector.tensor_tensor(alpha[:, :], sacc[:, :], cbias[:, :], ALU.add)
                    nc.vector.reciprocal(alpha[:, :], alpha[:, :])
                    al16 = amp.tile([128, NT], BF16, tag=f"al16{s}", name=f"al16{s}")
                    nc.vector.tensor_copy(al16[:, :], alpha[:, :])
                    nc.sync.dma_start(out=bnc_pview[s], in_=al16[:, :])
                    arow = amp.tile([1, T], BF16, tag="arow", name="arow")
                    nc.sync.dma_start(out=arow[0:1, :], in_=bounce[s:s + 1, :])

                    # transposes: sp (bf16) -> spt (bf16); copies 1/3 scalar, 2/3 vector
                    ncopy = 0
                    for jt in range(NT):
                        nit = NT - jt
                        for g0 in range(0, nit, 4):
                            gn = min(4, nit - g0)
                            tr = ppb.tile([128, 512], BF16, tag="trb", name="trb")
                            for gi in range(gn):
                                it = jt + g0 + gi
                                nc.tensor.transpose(tr[:, gi * 128:(gi + 1) * 128],
                                                    sp[it][:, jt * 128:(jt + 1) * 128],
                                                    identb[:, :])
                            if ncopy % 3 == 0:
                                nc.scalar.copy(spt[jt][:, g0 * 128:(g0 + gn) * 128], tr[:, 0:gn * 128])
                            else:
                                nc.vector.tensor_copy(spt[jt][:, g0 * 128:(g0 + gn) * 128], tr[:, 0:gn * 128])
                            ncopy += 1

                    # ---- one matvec pass: r = S'^T alpha (row form), bank-outer ----
                    ared = amp.tile([128, 1], F32, tag=f"ared{s}", name=f"ared{s}")
                    nc.vector.tensor_reduce(ared[:, :], alpha[:, :], axis=AXX, op=ALU.add)
                    ps1 = ppm.tile([1, 512], F32, tag="mm", name="mm")
                    _mm(ps1[0:1, 0:1], onescf[:, :], ared[:, :], start=True, stop=True)
                    asum = amp.tile([1, 1], F32, tag=f"asum{s}", name=f"asum{s}")
                    nc.scalar.copy(asum[0:1, :], ps1[0:1, 0:1])
                    brow = amp.tile([1, T], F32, tag="brow", name="brow")
                    bps = ppt.tile([128, 512], F32, tag="tr", name="tr")
                    for c4 in range(4):
                        lo, hi = c4 * 512, (c4 + 1) * 512
                        wps = ppw.tile([64, 512], F32, tag="wide", name="wide")
                        for it in range(4 * c4, NT):
                            L = (it + 1) * 128
                            shi = min(L, hi) - lo
                            _mm(wps[0:1, 0:shi], al16[:, it:it + 1], sp[it][:, lo:lo + shi],
                                start=(it == c4 * 4), stop=(it == NT - 1))
                        # brow holds r + sum(alpha); reciprocal happens in column space
                        nc.scalar.activation(brow[0:1, lo:hi], wps[0:1, :], AF.Identity,
                                             bias=asum[0:1, 0:1], scale=1.0)
                        for jq in range(4):
                            jt = 4 * c4 + jq
                            nc.tensor.transpose(bps[:, jt:jt + 1],
                                                brow[0:1, jt * 128:(jt + 1) * 128],
                                                ident[0:1, 0:1])
                    bcol = amp.tile([128, NT], F32, tag=f"bcol{s}", name=f"bcol{s}")
                    nc.scalar.copy(bcol[:, :], bps[:, 0:NT])
                    nc.vector.reciprocal(bcol[:, :], bcol[:, :])
                    bv = []
                    for jt in range(NT):
                        bvt = amp.tile([128, 64], BF16, tag=f"bv{jt}", name=f"bv{jt}")
                        nc.vector.tensor_scalar(bvt[:, :], vrow[s][:, jt * 64:(jt + 1) * 64],
                                                bcol[:, jt:jt + 1], None, ALU.mult)
                        bv.append(bvt)

                    # ---- y^T = alpha ∘ (S' @ (b∘V) + colsum(b∘V)), bank-outer ----
                    wcps = ppt.tile([128, 512], F32, tag="tr", name="tr")
                    for jt in range(NT):
                        _mm(wcps[0:1, 0:64], onescb[:, :], bv[jt][:, :],
                            start=(jt == 0), stop=(jt == NT - 1))
                    wrow = amp.tile([1, 64], F32, tag=f"wrow{s}", name=f"wrow{s}")
                    nc.scalar.copy(wrow[0:1, :], wcps[0:1, 0:64])
                    wtp = ppt.tile([128, 512], F32, tag="tr", name="tr")
                    nc.tensor.transpose(wtp[0:64, 0:1], wrow[0:1, :], ident[0:1, 0:1])
                    tw = amp.tile([64, 1], F32, tag=f"tw{s}", name=f"tw{s}")
                    nc.scalar.copy(tw[:, :], wtp[0:64, 0:1])
                    for c4 in range(4):
                        lo, hi = c4 * 512, (c4 + 1) * 512
                        sl = slice(lo, hi)
                        yps = ppw.tile([64, 512], F32, tag="wide", name="wide")
                        for jt in range(0, min(NT, 4 * c4 + 4)):
                            j0 = jt * 128
                            slo = max(lo, j0)
                            _mmb(yps[:, slo - lo:512], bv[jt][:, :],
                                 spt[jt][:, slo - j0:hi - j0],
                                 start=(jt == 0), stop=(jt == min(NT - 1, 4 * c4 + 3)))
                        psa = ppm.tile([128, 512], F32, tag="mm", name="mm")
                        _mm(psa[0:64, :], onesrb[0:1, 0:64], arow[0:1, sl], start=True, stop=True)
                        abc = amp.tile([64, 512], BF16, tag="abc", name="abc")
                        nc.scalar.copy(abc[:, :], psa[0:64, :])
                        ytmp = amp.tile([64, 512], BF16, tag="ytmp", name="ytmp")
                        nc.scalar.activation(ytmp[:, :], yps[:, :], AF.Identity,
                                             bias=tw[:, 0:1], scale=1.0)
                        nc.vector.tensor_tensor(ytmp[:, :], ytmp[:, :], abc[:, :], ALU.mult)
                        for grp in range(2):
                            nc.gpsimd.dma_start(out=a2a_in[s][grp * 4 + c4, :, :],
                                                in_=ytmp[:, :])

                    # per-slot AllToAll: slot 0's overlaps slot 1's compute
                    nc.gpsimd.collective_compute(
                        "AllToAll", ALU.bypass,
                        replica_groups=[list(range(N_CORES))],
                        ins=[a2a_in[s].opt()],
                        outs=[a2a_out[s].opt()],
                    )

            # ---------------- phase 5: proj + LN2 + MLP ----------------
            with tc.tile_pool(name="tail", bufs=1) as tp:
                stk0 = [tp.tile([128, 512], BF16, tag=f"stk0{h}", name=f"stk0{h}") for h in range(H)]
                for h in range(H):
                    c0, s0 = UNIT_SLOT[h]
                    c1_, s1_ = UNIT_SLOT[H + h]
                    nc.gpsimd.dma_start(out=stk0[h][0:64, :], in_=a2a_out[s0][c0, :, :])
                    nc.gpsimd.dma_start(out=stk0[h][64:128, :], in_=a2a_out[s1_][c1_, :, :])

                hT = [tp.tile([128, 512], F32R, tag=f"ht{ec}", name=f"ht{ec}") for ec in range(3)]
                for ec in range(3):
                    ps = ppm.tile([128, 512], F32, tag="mm", name="mm")
                    for h in range(H):
                        _mm(ps[:, :], wproj[h][ec], stk0[h][:, :],
                            start=(h == 0), stop=(h == H - 1))
                    nc.scalar.activation(hT[ec][:, :], ps[:, :], AF.Identity,
                                         bias=bproj[:, ec:ec + 1], scale=1.0)

                mu2ps = ppm.tile([1, 512], F32, tag="mm", name="mm")
                for ec in range(3):
                    _mm(mu2ps[0:1, :], onesc[:, :], hT[ec][:, :], start=(ec == 0), stop=(ec == 2))
                mu2r = tp.tile([1, 512], F32R, tag="mu2r", name="mu2r")
                nc.scalar.activation(mu2r[0:1, :], mu2ps[0:1, :], AF.Identity,
                                     bias=sbias[0:1, 0:1], scale=1.0 / CP1)
                scr2 = tp.tile([128, 512], F32R, tag="scr2", name="scr2")
                msq2ps = ppm.tile([1, 512], F32, tag="mm", name="mm")
                for ec in range(3):
                    nc.scalar.square(scr2[:, :], hT[ec][:, :])
                    _mm(msq2ps[0:1, :], onesc[:, :], scr2[:, :], start=(ec == 0), stop=(ec == 2))
                msq2r = tp.tile([1, 512], F32, tag="msq2r", name="msq2r")
                nc.scalar.activation(msq2r[0:1, :], msq2ps[0:1, :], AF.Identity,
                                     bias=sbias[0:1, 1:2], scale=1.0 / CP1)
                v2r = tp.tile([1, 512], F32R, tag="v2r", name="v2r")
                nc.vector.tensor_tensor(v2r[0:1, :], mu2r[0:1, :], mu2r[0:1, :], ALU.mult)
                nc.vector.tensor_tensor(v2r[0:1, :], msq2r[0:1, :], v2r[0:1, :], ALU.subtract)
                nc.scalar.activation(v2r[0:1, :], v2r[0:1, :], AF.Ln, bias=epsc[0:1, 0:1])
                # rstd2 = Exp(-0.5*ln(var+eps)) straight from the broadcast psum
                ps = ppm.tile([128, 512], F32, tag="mm", name="mm")
                _mm(ps[:, :], onesr[:, :], v2r[0:1, :], start=True, stop=True)
                rstd2bc = tp.tile([128, 512], F32, tag="rstd2bc", name="rstd2bc")
                nc.scalar.activation(rstd2bc[:, :], ps[:, :], AF.Exp, scale=-0.5)
                rstd2r = tp.tile([1, 512], F32R, tag="rstd2r", name="rstd2r")
                nc.vector.tensor_copy(rstd2r[0:1, :], rstd2bc[0:1, :])
                m2rr = tp.tile([1, 512], BF16, tag="m2rr", name="m2rr")
                b2rr = tp.tile([1, 512], BF16, tag="b2rr", name="b2rr")
                nc.vector.tensor_tensor(m2rr[0:1, :], mu2r[0:1, :], rstd2r[0:1, :], ALU.mult)
                b2f = tp.tile([1, 512], F32R, tag="b2f", name="b2f")
                nc.vector.tensor_scalar(b2f[0:1, :], mu2r[0:1, :], tcol[0:1, 0:1], None, ALU.subtract)
                nc.vector.tensor_tensor(b2rr[0:1, :], b2f[0:1, :], rstd2r[0:1, :], ALU.mult)
                hs = [tp.tile([128, 512], BF16, tag=f"hs{ec}", name=f"hs{ec}") for ec in range(3)]
                for ec in range(3):
                    nc.vector.tensor_tensor(hs[ec][:, :], hT[ec][:, :], rstd2bc[:, :], ALU.mult)

                mT = [tp.tile([128, 512], BF16, tag=f"mt{jc}", name=f"mt{jc}") for jc in range(12)]
                for jc in range(12):
                    ps = ppm.tile([128, 512], F32, tag="mm", name="mm")
                    for kc in range(3):
                        _mm(ps[:, :], wf[jc][kc], hs[kc][:, :], start=(kc == 0), stop=False)
                    _mm(ps[:, :], ns2f[0:1, jc * 128:(jc + 1) * 128], m2rr[0:1, :], start=False, stop=False)
                    _mm(ps[:, :], nwft[0:1, jc * 128:(jc + 1) * 128], b2rr[0:1, :], start=False, stop=True)
                    nc.scalar.activation(mT[jc][:, :], ps[:, :], AF.Gelu,
                                         bias=c2b[:, jc:jc + 1], scale=1.0)
                for ec in range(3):
                    ps = ppm.tile([128, 512], F32, tag="mm", name="mm")
                    for kc in range(12):
                        _mm(ps[:, :], wf2[ec][kc], mT[kc][:, :],
                            start=(kc == 0), stop=(kc == 11))
                    oT = tp.tile([128, 512], F32, tag=f"ot{ec}", name=f"ot{ec}")
                    nc.scalar.activation(oT[:, :], ps[:, :], AF.Identity,
                                         bias=bfc2[:, ec:ec + 1], scale=1.0)
                    nc.sync.dma_start(out=out_d[ec * 128:(ec + 1) * 128, :], in_=oT[:, :])

    nc.compile()
    return nc


def host_prep(inputs):
    x = np.asarray(inputs["x"], np.float32)
    t = float(np.asarray(inputs["t"]).reshape(-1)[0])
    w1 = np.asarray(inputs["ln1_w"], np.float32); b1 = np.asarray(inputs["ln1_b"], np.float32)
    Wa = np.asarray(inputs["attn_w"], np.float32); ba = np.asarray(inputs["attn_b"], np.float32)
    Wp_ = w1[:, None] * Wa
    c1 = b1 @ Wa + ba
    Wa_main, Wa_trow = Wp_[:C], Wp_[C]
    s1 = Wp_[:C].sum(axis=0)
    w2 = np.asarray(inputs["ln2_w"], np.float32); b2 = np.asarray(inputs["ln2_b"], np.float32)
    Wf = np.asarray(inputs["fc_w"], np.float32); bf = np.asarray(inputs["fc_b"], np.float32)
    Wf_p = w2[:, None] * Wf
    c2 = b2 @ Wf + bf
    Wf_main, Wf_trow = Wf_p[:C], Wf_p[C]
    s2f = Wf_p[:C].sum(axis=0)
    Wpj = np.asarray(inputs["proj_w"], np.float32); bpj = np.asarray(inputs["proj_b"], np.float32)
    Wf2 = np.asarray(inputs["fc2_w"], np.float32); bf2 = np.asarray(inputs["fc2_b"], np.float32)

    common = {
        "ident": np.eye(128, dtype=np.float32),
        "onesc": np.ones((128, 1), np.float32),
        "onesr": np.ones((1, 128), np.float32),
        "tcol": np.full((128, 1), t, np.float32),
        "sbias": np.array([[t / CP1, t * t / CP1]], np.float32),
        "epsc": np.full((128, 1), EPS, np.float32),
        "cbias": np.broadcast_to(
            np.array([T - (it + 1) * 128 for it in range(NT)], np.float32),
            (128, NT)).copy(),
        "bproj": bpj.reshape(3, 128).T.astype(np.float32).copy(),
        "c2b": c2.reshape(12, 128).T.astype(np.float32).copy(),
        "bfc2": bf2.reshape(3, 128).T.astype(np.float32).copy(),
        "nwft": (-Wf_trow)[None, :].astype(BF16NP).copy(),
        "ns2f": (-s2f)[None, :].astype(BF16NP).copy(),
        "wf": np.stack([np.stack([Wf_main[kc * 128:(kc + 1) * 128, jc * 128:(jc + 1) * 128]
                                  for kc in range(3)]) for jc in range(12)])
              .transpose(2, 0, 1, 3).reshape(128, -1).astype(BF16NP).copy(),
        "wf2": np.stack([np.stack([Wf2[kc * 128:(kc + 1) * 128, ec * 128:(ec + 1) * 128]
                                   for kc in range(12)]) for ec in range(3)])
               .transpose(2, 0, 1, 3).reshape(128, -1).astype(BF16NP).copy(),
    }

    in_maps = []
    for c in range(N_CORES):
        units = CORE_UNITS[c]
        myb = UNITS[units[0]][0]
        m = dict(common)
        m["xT"] = np.ascontiguousarray(x[myb].T)
        shard_b = c // 4  # batch of the row shard this core finishes (receiver side)
        wproj = np.zeros((H, 3, 128, 128), np.float32)
        for h in range(H):
            for ec in range(3):
                blk = Wpj[h * HD:(h + 1) * HD, ec * 128:(ec + 1) * 128]
                if shard_b == 0:
                    wproj[h, ec, 0:64] = blk
                else:
                    wproj[h, ec, 64:128] = blk
        m["wproj"] = wproj.transpose(2, 0, 1, 3).reshape(128, -1).astype(BF16NP).copy()
        wqk = np.zeros((2, 3, 128, 128), np.float32)
        r1qk = np.zeros((1, 512), np.float32)
        c1qk = np.zeros((128, 2), np.float32)
        wv = np.zeros((3, 128, 128), np.float32)
        r1v = np.zeros((1, 256), np.float32)
        c1v = np.zeros((128, 1), np.float32)
        for s, u in enumerate(units):
            _, h = UNITS[u]
            cq = slice(h * HD, (h + 1) * HD)
            ck = slice(C + h * HD, C + (h + 1) * HD)
            cv = slice(2 * C + h * HD, 2 * C + (h + 1) * HD)
            for kc in range(3):
                wqk[s, kc, :, 0:64] = Wa_main[kc * 128:(kc + 1) * 128, cq]
                wqk[s, kc, :, 64:128] = Wa_main[kc * 128:(kc + 1) * 128, ck]
                wv[kc, :, s * 64:(s + 1) * 64] = Wa_main[kc * 128:(kc + 1) * 128, cv]
            base = 2 * s * 128
            r1qk[0, base:base + 64] = -Wa_trow[cq]; r1qk[0, base + 64:base + 128] = -Wa_trow[ck]
            r1qk[0, base + 128:base + 192] = -s1[cq]; r1qk[0, base + 192:base + 256] = -s1[ck]
            r1v[0, s * 64:(s + 1) * 64] = -Wa_trow[cv]
            r1v[0, 128 + s * 64:128 + (s + 1) * 64] = -s1[cv]
            c1qk[0:64, s] = c1[cq]; c1qk[64:128, s] = c1[ck]
            c1v[s * 64:(s + 1) * 64, 0] = c1[cv]
        m["wqk"] = wqk; m["r1qk"] = r1qk; m["c1qk"] = c1qk
        m["wv"] = wv; m["r1v"] = r1v; m["c1v"] = c1v
        in_maps.append(m)
    return in_maps


def kernel(**inputs):
    if "nc" not in _COMPILED:
        _COMPILED["nc"] = build_program()
    nc = _COMPILED["nc"]
    in_maps = host_prep(inputs)
    res = run_bass_kernel_spmd(nc, in_maps, list(range(N_CORES)))
    out = np.zeros((B, T, C), np.float32)
    for c in range(N_CORES):
        oT = res.results[c]["oT"]
        b, t0 = c // 4, (c % 4) * 512
        out[b, t0:t0 + 512, :] = oT.T
    return out

